# revision 1
# baseline (speedup 1.0000x reference)
"""Multi-head attention forward on 8 Trainium2 NeuronCores (Bass/Tile).

Problem: x[4, 2048, 768] -> qkv proj (w_qkv[2304, 768]) -> 12-head attention
(softmax((q k^T) * 768^-0.5)) -> out proj (w_out[768, 768]).

Sharding: core c handles batch b = c//2 and a group of 6 heads g = c%2
(tensor parallel over heads within a batch pair). Each core computes a
partial output (its heads' contribution through the row-sliced out
projection, transposed: [768, 2048]); the host sums the two partials per
batch, transposes back and adds b_out.

Device-side layout notes (everything transposed so the contraction dim sits
on SBUF partitions):
  xT   [768, 2048]  built on-chip via PE transposes of x tiles
  qkvT [feat, 2048] = wT.T @ xT via fp32r matmuls (full-rate fp32)
  scoresT[keys, q]  = kT_tile.T @ qT  (so attn@v needs no transpose)
  softmax without max-subtraction (scores are O(1); exp is safe in fp32);
  denominator comes free from an appended ones-column in v ("v_aug"),
  divide folded into the PSUM->SBUF copyback on DVE.
"""

import os
import sys

import ml_dtypes
import numpy as np

_bf16 = ml_dtypes.bfloat16

if "/opt/trn_rl_repo" not in sys.path:
    sys.path.insert(0, "/opt/trn_rl_repo")

B = 4
N = 2048
DIM = 768
HEADS = 12
DHEAD = 64
SCALE = DIM ** (-0.5)
NCORES = 8
HPC = 6  # heads per core
FEAT = HPC * DHEAD  # 384 per-core attention features

_PROGRAM = None  # (nc,) cached compiled bass program


def _build_program():
    from contextlib import ExitStack

    import concourse.bass as bass
    import concourse.tile as tile
    from concourse import bacc, mybir
    from concourse.masks import make_identity

    f32 = mybir.dt.float32
    f32r = mybir.dt.float32r
    bf16 = mybir.dt.bfloat16
    Alu = mybir.AluOpType
    ActF = mybir.ActivationFunctionType

    nc = bacc.Bacc("TRN2", target_bir_lowering=False, debug=False)

    x_in = nc.dram_tensor("x", [N, DIM], bf16, kind="ExternalInput")
    wqkvT = nc.dram_tensor("wqkvT", [DIM, 3 * FEAT], bf16, kind="ExternalInput")
    bqkv = nc.dram_tensor("bqkv", [128, 9], f32, kind="ExternalInput")
    woutT = nc.dram_tensor("woutT", [FEAT, DIM], f32r, kind="ExternalInput")
    out_T = nc.dram_tensor("outT", [DIM, N], f32, kind="ExternalOutput")

    NT = N // 128  # 16 n-tiles
    KC = DIM // 128  # 6 contraction chunks for dim
    NSPAN = N // 512  # 4 moving spans

    with tile.TileContext(nc) as tc, ExitStack() as ctx:
        const = ctx.enter_context(tc.tile_pool(name="const", bufs=1))
        identity_bf = const.tile([128, 128], bf16)
        make_identity(nc, identity_bf)
        ones_f32 = const.tile([128, 1], f32)
        nc.vector.memset(ones_f32[:, :], 1.0)
        ones65 = const.tile([65, 64], f32r)
        nc.vector.tensor_copy(
            out=ones65[:, :], in_=ones_f32[0:65, :].to_broadcast((65, 64))
        )
        bias_sb = const.tile([128, 9], f32)
        nc.gpsimd.dma_start(bias_sb[:, :], bqkv[:, :])

        # ---- Phase 1: x loads (bf16) + PE transposes into xT ----
        xt_pool = ctx.enter_context(tc.tile_pool(name="xT", bufs=1))
        xT = xt_pool.tile([128, KC, N], bf16)
        xin_pool = ctx.enter_context(tc.tile_pool(name="xin", bufs=4))
        xins = []
        for i in range(NT):
            xin = xin_pool.tile([128, DIM], bf16, name="xin", tag="xin", bufs=None)
            xins.append(xin)
        # x-tile DMAs first (first 8 gate the first qkv chains), then weights
        for i in range(8):
            eng = nc.gpsimd if i % 2 == 0 else nc.sync
            eng.dma_start(xins[i][:, :], x_in[i * 128 : (i + 1) * 128, :])

        wpool = ctx.enter_context(tc.tile_pool(name="w", bufs=1))
        w_all = wpool.tile([128, KC, 3 * FEAT], bf16)
        for j in range(KC):
            nc.gpsimd.dma_start(w_all[:, j, :], wqkvT[j * 128 : (j + 1) * 128, :])
        wout_sb = wpool.tile([128, 3, DIM], f32r)
        for c in range(3):
            nc.gpsimd.dma_start(wout_sb[:, c, :], woutT[c * 128 : (c + 1) * 128, :])
        for i in range(8, NT):
            eng = nc.gpsimd if i % 2 == 0 else nc.sync
            eng.dma_start(xins[i][:, :], x_in[i * 128 : (i + 1) * 128, :])

        # PSUM: spool 3x[128,2,512] (6 banks) + opool 2x[128,512] (2 banks)
        spool = ctx.enter_context(tc.tile_pool(name="spsum", bufs=3, space="PSUM"))
        opool = ctx.enter_context(tc.tile_pool(name="opsum", bufs=2, space="PSUM"))

        qk_pool = ctx.enter_context(tc.tile_pool(name="qk", bufs=2))
        vt_pool = ctx.enter_context(tc.tile_pool(name="vt", bufs=2))
        vaug_pool = ctx.enter_context(tc.tile_pool(name="vaug", bufs=2))
        exp_pool = ctx.enter_context(tc.tile_pool(name="expT", bufs=6))
        rcp_pool = ctx.enter_context(tc.tile_pool(name="rcp", bufs=2))
        sbo_pool = ctx.enter_context(tc.tile_pool(name="sbo", bufs=3))
        hst_pool = ctx.enter_context(tc.tile_pool(name="hstage", bufs=2))
        ao_pool = ctx.enter_context(tc.tile_pool(name="attnout", bufs=1))
        attn_outT = ao_pool.tile([128, 3, N], f32r)
        ost_pool = ctx.enter_context(tc.tile_pool(name="ostage", bufs=3))

        def emit_xtrans(i):
            for j in range(KC):
                tp = spool.tile([128, 128], bf16, tag="s", name="tp_x")
                nc.tensor.transpose(
                    tp[:, :], xins[i][:, j * 128 : (j + 1) * 128], identity_bf[:, :]
                )
                nc.vector.tensor_copy(
                    out=xT[:, j, i * 128 : (i + 1) * 128], in_=tp[:, :]
                )

        def emit_qkv_chain(qk_t, vT_t, hp, idx, m, sp2):
            """One (m, span-pair): 12 matmuls + biased copyback."""
            ps = spool.tile([128, 2, 512], f32, tag="s", name="ps_qkv")
            for u in range(2):
                span = 2 * sp2 + u
                for j in range(KC):
                    nc.tensor.matmul(
                        ps[:, u, :],
                        w_all[:, j, m * 128 : (m + 1) * 128],
                        xT[:, j, span * 512 : (span + 1) * 512],
                        start=(j == 0),
                        stop=(j == KC - 1),
                    )
            cols = slice(sp2 * 1024, (sp2 + 1) * 1024)
            if idx < 2:
                # head A -> chunk idx rows 0:64; head B -> chunk idx+2 rows 64:128
                nc.vector.tensor_scalar(
                    qk_t[0:64, idx, cols].rearrange("p (a b) -> p a b", a=2),
                    ps[0:64, :, :],
                    bias_sb[0:64, m : m + 1],
                    None,
                    Alu.add,
                )
                nc.vector.tensor_scalar(
                    qk_t[64:128, idx + 2, cols].rearrange("p (a b) -> p a b", a=2),
                    ps[64:128, :, :],
                    bias_sb[64:128, m : m + 1],
                    None,
                    Alu.add,
                )
            else:
                nc.vector.tensor_scalar(
                    vT_t[:, cols].rearrange("p (a b) -> p a b", a=2),
                    ps[:, :, :],
                    bias_sb[:, m : m + 1],
                    None,
                    Alu.add,
                )

        def emit_vtrans(vaug_t, vT_t, kc0, kcn):
            for kc in range(kc0, kcn):
                tp = spool.tile([128, 128], bf16, tag="s", name="tp_v")
                nc.tensor.transpose(
                    tp[:, :], vT_t[:, kc * 128 : (kc + 1) * 128], identity_bf[:, :]
                )
                nc.vector.tensor_copy(
                    out=vaug_t[:, kc, :].rearrange("p (t c) -> p t c", t=2)[
                        :, :, 0:64
                    ],
                    in_=tp[:, :].rearrange("p (t c) -> p t c", t=2),
                )


        def make_pair_units(hp):
            """Allocate tiles + return (tiles, list of PE filler closures)."""
            qk_t = qk_pool.tile([128, 4, N], bf16, name="qk", tag="qk")
            vT_t = vt_pool.tile([128, N], bf16, name="vT", tag="vT")
            vaug_t = vaug_pool.tile([128, NT, 256], bf16, name="vaug", tag="vaug")
            units = []

            def zero_pads():
                # zero the unused halves so K/M padding contributes nothing
                nc.gpsimd.memset(qk_t[64:128, 0:2, :], 0.0)
                nc.gpsimd.memset(qk_t[0:64, 2:4, :], 0.0)
                nc.gpsimd.memset(vaug_t[:, :, :], 0.0)

            units.append(zero_pads)
            order = [
                (0, hp, 0),
                (1, 3 + hp, 0),
                (2, 6 + hp, 0),
                (0, hp, 1),
                (1, 3 + hp, 1),
                (2, 6 + hp, 1),
            ]
            for idx, m, sp2 in order:
                units.append(
                    lambda i=idx, mm=m, s=sp2: emit_qkv_chain(qk_t, vT_t, hp, i, mm, s)
                )

            def vaug_init():
                ones_cols = vaug_t[:, :, :].rearrange("p k (t c) -> p k t c", t=2)[
                    :, :, :, 64:65
                ]
                nc.vector.tensor_copy(
                    out=ones_cols, in_=ones_f32[:, :].to_broadcast((128, NT, 2, 1))
                )
                emit_vtrans(vaug_t, vT_t, 0, 4)

            units.append(vaug_init)
            for kc0 in (4, 8, 12):
                units.append(lambda k=kc0: emit_vtrans(vaug_t, vT_t, k, k + 4))
            return (qk_t, vT_t, vaug_t), units

        def emit_outproj(m, span):
            """One out-proj tile [128, 512]: 3 matmuls + copyback + DMA."""
            ps = spool.tile([128, 2, 512], f32, tag="s", name="ps_op")
            for c in range(3):
                nc.tensor.matmul(
                    ps[:, 0, :],
                    wout_sb[:, c, m * 128 : (m + 1) * 128],
                    attn_outT[:, c, span * 512 : (span + 1) * 512],
                    start=(c == 0),
                    stop=(c == 2),
                )
            ostage = ost_pool.tile([128, 512], f32, name="ostage", tag="ostage")
            nc.vector.tensor_copy(out=ostage[:, :], in_=ps[:, 0, :])
            nc.gpsimd.dma_start(
                out_T[m * 128 : (m + 1) * 128, span * 512 : (span + 1) * 512],
                ostage[:, :],
            )

        # ---- phase 1 transposes + qkv/vaug for head pair 0, interleaved ----
        cur_tiles, units0 = make_pair_units(0)
        for i in range(8):
            emit_xtrans(i)
        for u in units0[0:3]:  # sp2=0 chains
            u()
        for i in range(8, NT):
            emit_xtrans(i)
        for u in units0[3:]:  # sp2=1 chains + vaug units
            u()

        # ---- attention per head pair, interleaving filler PE work ----
        for hp in range(3):
            qk, vT, vaug = cur_tiles
            if hp < 2:
                cur_tiles, filler = make_pair_units(hp + 1)
                fill_stride = max(1, (64 + len(filler)) // (len(filler) + 1))
            else:
                filler = []  # outproj units appended dynamically by normalize
                fill_stride = 1
            half_ctr = 0
            pending = [None]  # deferred normalize closure

            def flush_pending():
                if pending[0] is not None:
                    pending[0]()
                    pending[0] = None

            for j in range(2):
                qT = qk[:, 2 * j, :]
                kT = qk[:, 2 * j + 1, :]
                for span in range(NSPAN):
                    po = opool.tile([128, 512], f32, tag="o", name="po")
                    ets = []
                    for half in range(8):
                        ps = spool.tile([128, 2, 512], f32, tag="s", name="ps_s")
                        for u in range(2):
                            kc = 2 * half + u
                            nc.tensor.matmul(
                                ps[:, u, :],
                                kT[:, kc * 128 : (kc + 1) * 128],
                                qT[:, span * 512 : (span + 1) * 512],
                                start=True,
                                stop=True,
                            )
                        et = exp_pool.tile([128, 2, 512], bf16)
                        nc.scalar.activation(
                            et[:, :, :], ps[:, :, :], ActF.Exp, scale=float(SCALE)
                        )
                        ets.append(et)
                        if half == 1:
                            flush_pending()
                        if half >= 1:
                            pet = ets[half - 1]
                            for u in range(2):
                                kc = 2 * (half - 1) + u
                                nc.tensor.matmul(
                                    po[:, :],
                                    vaug[:, kc, j * 128 : (j + 1) * 128],
                                    pet[:, u, :],
                                    start=(kc == 0),
                                    stop=False,
                                )
                        half_ctr += 1
                        if hp == 2:
                            if len(filler) > 6:
                                filler.pop(0)()
                        elif filler and fill_stride and half_ctr % fill_stride == 0:
                            filler.pop(0)()
                    pet = ets[7]
                    for u in range(2):
                        kc = 14 + u
                        nc.tensor.matmul(
                            po[:, :],
                            vaug[:, kc, j * 128 : (j + 1) * 128],
                            pet[:, u, :],
                            start=False,
                            stop=(kc == 15),
                        )
                    # denominator recip + PSUM->SBUF copy issued immediately
                    # so the deferred broadcast matmul never waits on DVE
                    rs = rcp_pool.tile([65, 512], f32r, name="rs")
                    with nc.allow_low_precision(reason="fp32r recip"):
                        nc.vector.reciprocal(rs[64:65, :], po[64:65, :])
                    sb_o = sbo_pool.tile([65, 512], f32, name="sb_o")
                    nc.vector.tensor_copy(out=sb_o[:, :], in_=po[0:65, :])

                    def normalize(j=j, span=span, po=po, hp=hp, rs=rs, sb_o=sb_o):
                        # PE-broadcast of the recip back into po's own bank
                        # (WAR-ordered after the early copy), then divide.
                        nc.tensor.matmul(
                            po[0:64, :],
                            ones65[64:65, :],
                            rs[64:65, :],
                            start=True,
                            stop=True,
                        )
                        if j == 0:
                            ddst = attn_outT[0:64, hp, span * 512 : (span + 1) * 512]
                        else:
                            ddst = hst_pool.tile(
                                [64, 512], f32r, name="hstage", tag="hstage"
                            )
                        nc.vector.tensor_tensor(
                            out=ddst,
                            in0=sb_o[0:64, :],
                            in1=po[0:64, :],
                            op=Alu.mult,
                        )
                        if j == 1:
                            nc.gpsimd.dma_start(
                                attn_outT[64:128, hp, span * 512 : (span + 1) * 512],
                                ddst[:, :],
                            )
                            if hp == 2:
                                for m in range(DIM // 128):
                                    filler.append(
                                        lambda mm=m, s=span: emit_outproj(mm, s)
                                    )

                    pending[0] = normalize
            flush_pending()
            while filler:
                filler.pop(0)()

    nc.compile()
    return nc


def _get_program():
    global _PROGRAM
    if _PROGRAM is None:
        _PROGRAM = _build_program()
    return _PROGRAM


def _round_to_f32r(a):
    """Round fp32 to the PE's fp32r format: 11-bit mantissa, low 12 bits zero
    (round to nearest, ties away handled approximately via +0x7FF + lsb)."""
    u = np.ascontiguousarray(a, dtype=np.float32).view(np.uint32)
    r = u + np.uint32(0x7FF) + ((u >> np.uint32(12)) & np.uint32(1))
    r &= np.uint32(0xFFFFF000)
    return r.view(np.float32)


def make_core_inputs(x, w_qkv, b_qkv, w_out):
    """Host-side shard: per-core input dicts for cores 0..7."""
    x = np.asarray(x, dtype=np.float32)
    w_qkv = np.asarray(w_qkv, dtype=np.float32)
    b_qkv = np.asarray(b_qkv, dtype=np.float32)
    w_out = np.asarray(w_out, dtype=np.float32)

    per_group = []
    for g in range(2):
        rows = np.concatenate(
            [
                w_qkv[qkv * DIM + g * FEAT : qkv * DIM + (g + 1) * FEAT]
                for qkv in range(3)
            ],
            axis=0,
        )  # [1152, 768]
        wqkvT_g = np.ascontiguousarray(rows.T).astype(_bf16)  # [768, 1152]
        b_rows = np.concatenate(
            [
                b_qkv[qkv * DIM + g * FEAT : qkv * DIM + (g + 1) * FEAT]
                for qkv in range(3)
            ],
            axis=0,
        )  # [1152]
        bias_g = np.ascontiguousarray(b_rows.reshape(9, 128).T)  # [128, 9]
        woutT_g = _round_to_f32r(w_out[:, g * FEAT : (g + 1) * FEAT].T)
        per_group.append((wqkvT_g, bias_g, woutT_g))

    x_bf = [np.ascontiguousarray(x[b]).astype(_bf16) for b in range(B)]
    in_maps = []
    for c in range(NCORES):
        b, g = c // 2, c % 2
        wqkvT_g, bias_g, woutT_g = per_group[g]
        in_maps.append(
            {
                "x": x_bf[b],
                "wqkvT": wqkvT_g,
                "bqkv": bias_g,
                "woutT": woutT_g,
            }
        )
    return in_maps


def assemble_output(results, b_out):
    """Host-side unshard: sum partials per batch pair, transpose, add bias."""
    b_out = np.asarray(b_out, dtype=np.float32)
    out = np.empty((B, N, DIM), dtype=np.float32)
    for b in range(B):
        pT = results[2 * b]["outT"] + results[2 * b + 1]["outT"]  # [768, 2048]
        out[b] = pT.T + b_out[None, :]
    return out


def kernel(x, w_qkv, b_qkv, w_out, b_out):
    from concourse.bass_utils import run_bass_kernel_spmd

    nc = _get_program()
    in_maps = make_core_inputs(x, w_qkv, b_qkv, w_out)
    res = run_bass_kernel_spmd(nc, in_maps, list(range(NCORES)))
    return assemble_output(res.results, b_out)



# revision 8
# speedup vs baseline: 1.0625x; 1.0625x over previous
"""Multi-head attention forward on 8 Trainium2 NeuronCores (Bass/Tile).

Problem: x[4, 2048, 768] -> qkv proj (w_qkv[2304, 768]) -> 12-head attention
(softmax((q k^T) * 768^-0.5)) -> out proj (w_out[768, 768]).

Sharding: core c handles batch b = c//2 and a group of 6 heads g = c%2
(tensor parallel over heads within a batch pair). Each core computes a
partial output (its heads' contribution through the row-sliced out
projection, transposed: [768, 2048]); the host sums the two partials per
batch, transposes back and adds b_out.

Device-side layout notes (everything transposed so the contraction dim sits
on SBUF partitions):
  xT   [768, 2048]  transposed on the HOST, plain DMA load (no PE transposes)
  qkvT [feat, 2048] = wT.T @ xT via bf16 matmuls
  qk_t [128, 4, N]: head A of the pair in rows 0:64 of chunks 0 (q) / 1 (k),
  head B in rows 64:128 of chunks 2/3; unused halves zeroed so the K=128
  scores contraction sees zero padding.
  scoresT[keys, q]  = kT_tile.T @ qT  (so attn@v needs no transpose)
  softmax without max-subtraction (scores are O(1); exp is safe in fp32);
  denominator comes free from an appended ones-column in v ("v_aug").
  Normalize: reciprocal_approx_fast on DVE, bf16 PE broadcast of the recip
  (deferred into the next block so it never stalls PE), DVE multiply.

Schedule: span-major blocks of 8 half-slots (2 score matmuls + 1 exp each);
attn@v trails its exp by LAG half-slots; qkv chains / v transposes for the
next head pair and out-proj tiles drain from a debt-paced filler queue so
the PE never idles while the ACT engine chews exps.
"""

import os
import sys

import ml_dtypes
import numpy as np

_bf16 = ml_dtypes.bfloat16

if "/opt/trn_rl_repo" not in sys.path:
    sys.path.insert(0, "/opt/trn_rl_repo")

B = 4
N = 2048
DIM = 768
HEADS = 12
DHEAD = 64
SCALE = DIM ** (-0.5)
NCORES = 8
HPC = 6  # heads per core
FEAT = HPC * DHEAD  # 384 per-core attention features

_PROGRAM = None  # (nc,) cached compiled bass program


def _build_program():
    from contextlib import ExitStack

    import concourse.bass as bass
    import concourse.tile as tile
    from concourse import bacc, mybir
    from concourse.masks import make_identity

    f32 = mybir.dt.float32
    f32r = mybir.dt.float32r
    bf16 = mybir.dt.bfloat16
    Alu = mybir.AluOpType
    ActF = mybir.ActivationFunctionType

    nc = bacc.Bacc("TRN2", target_bir_lowering=False, debug=False)

    x_in = nc.dram_tensor("xT", [DIM, N], bf16, kind="ExternalInput")
    wqkvT = nc.dram_tensor("wqkvT", [DIM, 3 * FEAT], bf16, kind="ExternalInput")
    bqkv = nc.dram_tensor("bqkv", [128, 9], f32, kind="ExternalInput")
    woutT = nc.dram_tensor("woutT", [FEAT, DIM], f32r, kind="ExternalInput")
    out_T = nc.dram_tensor("outT", [DIM, N], f32, kind="ExternalOutput")

    NT = N // 128  # 16 key tiles
    KC = DIM // 128  # 6 contraction chunks for dim
    NSPAN = N // 512  # 4 moving spans

    HALF_BUDGET = 300.0  # ns of filler debt accrued per half-slot
    DEBT_CAP = 6000.0
    CHAIN_COST = 2600.0
    VTRANS_COST = 900.0
    VAUGI_COST = 1100.0
    OUTPROJ_COST = 750.0
    ZPAD_COST = 100.0  # gpsimd memsets, nearly free for the PE
    LAG = 2  # attnv trails its scores by this many half-slots

    with tile.TileContext(nc) as tc, ExitStack() as ctx:
        const = ctx.enter_context(tc.tile_pool(name="const", bufs=1))
        identity_bf = const.tile([128, 128], bf16)
        make_identity(nc, identity_bf)
        ones_f32 = const.tile([128, 1], f32)
        nc.vector.memset(ones_f32[:, :], 1.0)
        ones_bf = const.tile([65, 64], bf16)
        nc.vector.tensor_copy(
            out=ones_bf[:, :], in_=ones_f32[0:65, :].to_broadcast((65, 64))
        )
        bias_sb = const.tile([128, 9], f32)
        nc.gpsimd.dma_start(bias_sb[:, :], bqkv[:, :])

        # ---- xT: transposed on host, plain DMA ----
        xt_pool = ctx.enter_context(tc.tile_pool(name="xT", bufs=1))
        xT = xt_pool.tile([128, KC, N], bf16)
        for j in range(KC):
            eng = nc.gpsimd if j % 2 == 0 else nc.sync
            eng.dma_start(xT[:, j, :], x_in[j * 128 : (j + 1) * 128, :])

        wpool = ctx.enter_context(tc.tile_pool(name="w", bufs=1))
        w_all = wpool.tile([128, KC, 3 * FEAT], bf16)
        for j in range(KC):
            eng = nc.gpsimd if j % 2 == 0 else nc.sync
            eng.dma_start(w_all[:, j, :], wqkvT[j * 128 : (j + 1) * 128, :])
        wout_sb = wpool.tile([128, 3, DIM], f32r)
        for c in range(3):
            nc.sync.dma_start(wout_sb[:, c, :], woutT[c * 128 : (c + 1) * 128, :])

        # PSUM: spool 3x[128,2,512] (6 banks) + opool 2x[128,512] (2 banks)
        spool = ctx.enter_context(tc.tile_pool(name="spsum", bufs=3, space="PSUM"))
        opool = ctx.enter_context(tc.tile_pool(name="opsum", bufs=2, space="PSUM"))

        qk_pool = ctx.enter_context(tc.tile_pool(name="qk", bufs=2))
        vt_pool = ctx.enter_context(tc.tile_pool(name="vt", bufs=2))
        vaug_pool = ctx.enter_context(tc.tile_pool(name="vaug", bufs=2))
        exp_pool = ctx.enter_context(tc.tile_pool(name="expT", bufs=6))
        rcpf_pool = ctx.enter_context(tc.tile_pool(name="rcpf", bufs=2))
        rcpb_pool = ctx.enter_context(tc.tile_pool(name="rcpb", bufs=2))
        sbo_pool = ctx.enter_context(tc.tile_pool(name="sbo", bufs=3))
        hst_pool = ctx.enter_context(tc.tile_pool(name="hstage", bufs=2))
        ao_pool = ctx.enter_context(tc.tile_pool(name="attnout", bufs=1))
        attn_outT = ao_pool.tile([128, 3, N], f32r)
        ost_pool = ctx.enter_context(tc.tile_pool(name="ostage", bufs=3))

        def emit_qkv_chain(qk_t, vT_t, idx, m, sp2):
            """One (m, span-pair): 12 matmuls + biased copyback."""
            ps = spool.tile([128, 2, 512], f32, tag="s", name="ps_qkv")
            for u in range(2):
                span = 2 * sp2 + u
                for j in range(KC):
                    nc.tensor.matmul(
                        ps[:, u, :],
                        w_all[:, j, m * 128 : (m + 1) * 128],
                        xT[:, j, span * 512 : (span + 1) * 512],
                        start=(j == 0),
                        stop=(j == KC - 1),
                    )
            cols = slice(sp2 * 1024, (sp2 + 1) * 1024)
            if idx < 2:
                # head A -> chunk idx rows 0:64; head B -> chunk idx+2 rows 64:128
                nc.vector.tensor_scalar(
                    qk_t[0:64, idx, cols].rearrange("p (a b) -> p a b", a=2),
                    ps[0:64, :, :],
                    bias_sb[0:64, m : m + 1],
                    None,
                    Alu.add,
                )
                nc.vector.tensor_scalar(
                    qk_t[64:128, idx + 2, cols].rearrange("p (a b) -> p a b", a=2),
                    ps[64:128, :, :],
                    bias_sb[64:128, m : m + 1],
                    None,
                    Alu.add,
                )
            else:
                nc.vector.tensor_scalar(
                    vT_t[:, cols].rearrange("p (a b) -> p a b", a=2),
                    ps[:, :, :],
                    bias_sb[:, m : m + 1],
                    None,
                    Alu.add,
                )

        def emit_zero_pads(qk_t, vaug_t):
            # zero the unused halves so K/M padding contributes nothing
            nc.gpsimd.memset(qk_t[64:128, 0:2, :], 0.0)
            nc.gpsimd.memset(qk_t[0:64, 2:4, :], 0.0)
            nc.gpsimd.memset(vaug_t[:, :, :], 0.0)

        def emit_vtrans(vaug_t, vT_t, kc0, kcn):
            for kc in range(kc0, kcn):
                tp = spool.tile([128, 128], bf16, tag="s", name="tp_v")
                nc.tensor.transpose(
                    tp[:, :], vT_t[:, kc * 128 : (kc + 1) * 128], identity_bf[:, :]
                )
                nc.vector.tensor_copy(
                    out=vaug_t[:, kc, :].rearrange("p (t c) -> p t c", t=2)[
                        :, :, 0:64
                    ],
                    in_=tp[:, :].rearrange("p (t c) -> p t c", t=2),
                )

        def emit_vaug_init(vaug_t, vT_t):
            ones_cols = vaug_t[:, :, :].rearrange("p k (t c) -> p k t c", t=2)[
                :, :, :, 64:65
            ]
            nc.vector.tensor_copy(
                out=ones_cols, in_=ones_f32[:, :].to_broadcast((128, NT, 2, 1))
            )
            emit_vtrans(vaug_t, vT_t, 0, 4)

        def emit_outproj(m, span):
            """One out-proj tile [128, 512]: 3 matmuls + copyback + DMA."""
            ps = spool.tile([128, 2, 512], f32, tag="s", name="ps_op")
            for c in range(3):
                nc.tensor.matmul(
                    ps[:, 0, :],
                    wout_sb[:, c, m * 128 : (m + 1) * 128],
                    attn_outT[:, c, span * 512 : (span + 1) * 512],
                    start=(c == 0),
                    stop=(c == 2),
                )
            ostage = ost_pool.tile([128, 512], f32, name="ostage", tag="ostage")
            nc.vector.tensor_copy(out=ostage[:, :], in_=ps[:, 0, :])
            nc.gpsimd.dma_start(
                out_T[m * 128 : (m + 1) * 128, span * 512 : (span + 1) * 512],
                ostage[:, :],
            )

        # ---- filler machinery: (cost, closure) FIFO drained by debt ----
        filler_q = []
        debt = [2000.0]

        def pump(amount):
            debt[0] = min(debt[0] + amount, DEBT_CAP)
            while filler_q and debt[0] >= filler_q[0][0]:
                c, fn = filler_q.pop(0)
                fn()
                debt[0] -= c

        def make_pair_tiles():
            qk_t = qk_pool.tile([128, 4, N], bf16, name="qk", tag="qk")
            vT_t = vt_pool.tile([128, N], bf16, name="vT", tag="vT")
            vaug_t = vaug_pool.tile([128, NT, 256], bf16, name="vaug", tag="vaug")
            return qk_t, vT_t, vaug_t

        def push_pair_fillers(hp, qk_t, vT_t, vaug_t):
            """Queue pair hp's prep work as fillers for the prior pair."""
            filler_q.append((ZPAD_COST, lambda: emit_zero_pads(qk_t, vaug_t)))
            for idx, m, sp2 in (
                (0, hp, 0),
                (1, 3 + hp, 0),
                (1, 3 + hp, 1),
                (2, 6 + hp, 0),
                (2, 6 + hp, 1),
                (0, hp, 1),
            ):
                filler_q.append(
                    (
                        CHAIN_COST,
                        lambda i=idx, mm=m, s=sp2: emit_qkv_chain(qk_t, vT_t, i, mm, s),
                    )
                )
            filler_q.append((VAUGI_COST, lambda: emit_vaug_init(vaug_t, vT_t)))
            for kc0 in (4, 8, 12):
                filler_q.append(
                    (VTRANS_COST, lambda k=kc0: emit_vtrans(vaug_t, vT_t, k, k + 4))
                )

        # ---- attention block machinery ----
        pending = []  # closures popped with LAG half-slots of delay

        def mk_attnv(po, vaug_t, j, et, half):
            def go():
                for u in range(2):
                    kc = 2 * half + u
                    nc.tensor.matmul(
                        po[:, :],
                        vaug_t[:, kc, j * 128 : (j + 1) * 128],
                        et[:, u, :],
                        start=(kc == 0),
                        stop=(kc == 15),
                    )

            return go

        def emit_block(hp, j, span, qk_t, vaug_t, carry_flush):
            po = opool.tile([128, 512], f32, tag="o", name="po")
            qT = qk_t[:, 2 * j, span * 512 : (span + 1) * 512]
            for half in range(8):
                ps = spool.tile([128, 2, 512], f32, tag="s", name="ps_s")
                for u in range(2):
                    kc = 2 * half + u
                    nc.tensor.matmul(
                        ps[:, u, :],
                        qk_t[:, 2 * j + 1, kc * 128 : (kc + 1) * 128],
                        qT,
                        start=True,
                        stop=True,
                    )
                et = exp_pool.tile([128, 2, 512], bf16)
                nc.scalar.activation(
                    et[:, :, :], ps[:, :, :], ActF.Exp, scale=float(SCALE)
                )
                pending.append(mk_attnv(po, vaug_t, j, et, half))
                if half == 3 and carry_flush is not None:
                    pending.append(carry_flush)
                    carry_flush = None
                pump(HALF_BUDGET)
                while len(pending) > LAG:
                    pending.pop(0)()
            # post-block DVE chain: snapshot numerators+denominator, recip
            sb_o = sbo_pool.tile([65, 512], f32, name="sb_o")
            rcp_f = rcpf_pool.tile([65, 512], f32, name="rcp_f")
            rcp_b = rcpb_pool.tile([65, 512], bf16, name="rcp_b")

            def post_dve(po=po, sb_o=sb_o, rcp_f=rcp_f, rcp_b=rcp_b):
                nc.vector.tensor_copy(out=sb_o[:, :], in_=po[0:65, :])
                # NOTE: reciprocal_approx_fast mis-executes on HW (garbage
                # out, sim models it fine) — use the exact DVE reciprocal;
                # its ~3.3us is hidden by the deferred flush.
                nc.vector.reciprocal(rcp_f[64:65, :], sb_o[64:65, :])
                nc.vector.tensor_copy(out=rcp_b[64:65, :], in_=rcp_f[64:65, :])

            pending.append(post_dve)

            def flush(po=po, sb_o=sb_o, rcp_b=rcp_b, hp=hp, j=j, span=span):
                # PE-broadcast of the recip back into po's own bank
                # (WAR-ordered after the snapshot copy), then multiply.
                nc.tensor.matmul(
                    po[0:64, :],
                    ones_bf[64:65, 0:64],
                    rcp_b[64:65, :],
                    start=True,
                    stop=True,
                )
                if j == 0:
                    ddst = attn_outT[0:64, hp, span * 512 : (span + 1) * 512]
                else:
                    ddst = hst_pool.tile([64, 512], f32r, name="hstage", tag="hstage")
                nc.vector.tensor_tensor(
                    out=ddst, in0=sb_o[0:64, :], in1=po[0:64, :], op=Alu.mult
                )
                if j == 1:
                    nc.gpsimd.dma_start(
                        attn_outT[64:128, hp, span * 512 : (span + 1) * 512],
                        ddst[:, :],
                    )
                    if hp == 2:
                        for m in range(DIM // 128):
                            filler_q.append(
                                (OUTPROJ_COST, lambda mm=m, s=span: emit_outproj(mm, s))
                            )

            return flush

        # ---- prologue: pair-0 q/k/v chains ----
        qk0, vT0, vaug0 = make_pair_tiles()
        emit_zero_pads(qk0, vaug0)
        emit_qkv_chain(qk0, vT0, 0, 0, 0)  # q sp0
        emit_qkv_chain(qk0, vT0, 1, 3, 0)  # k sp0
        emit_qkv_chain(qk0, vT0, 1, 3, 1)  # k sp1
        emit_qkv_chain(qk0, vT0, 2, 6, 0)  # v sp0
        emit_qkv_chain(qk0, vT0, 2, 6, 1)  # v sp1
        emit_qkv_chain(qk0, vT0, 0, 0, 1)  # q sp1
        emit_vaug_init(vaug0, vT0)
        for kc0 in (4, 8, 12):
            filler_q.append(
                (VTRANS_COST, lambda k=kc0, v=vaug0, t=vT0: emit_vtrans(v, t, k, k + 4))
            )

        # ---- attention: 3 pairs x 4 spans x 2 heads, span-major ----
        cur = (qk0, vT0, vaug0)
        carry = None
        for hp in range(3):
            qk_t, vT_t, vaug_t = cur
            if hp < 2:
                nxt = make_pair_tiles()
                push_pair_fillers(hp + 1, *nxt)
                cur = nxt
            for span in range(NSPAN):
                for j in range(2):
                    carry = emit_block(hp, j, span, qk_t, vaug_t, carry)
        while pending:
            pending.pop(0)()
        carry()
        while filler_q:
            filler_q.pop(0)[1]()

    nc.compile()
    return nc


def _get_program():
    global _PROGRAM
    if _PROGRAM is None:
        _PROGRAM = _build_program()
    return _PROGRAM


def _round_to_f32r(a):
    """Round fp32 to the PE's fp32r format: 11-bit mantissa, low 12 bits zero
    (round to nearest, ties away handled approximately via +0x7FF + lsb)."""
    u = np.ascontiguousarray(a, dtype=np.float32).view(np.uint32)
    r = u + np.uint32(0x7FF) + ((u >> np.uint32(12)) & np.uint32(1))
    r &= np.uint32(0xFFFFF000)
    return r.view(np.float32)


def make_core_inputs(x, w_qkv, b_qkv, w_out):
    """Host-side shard: per-core input dicts for cores 0..7."""
    x = np.asarray(x, dtype=np.float32)
    w_qkv = np.asarray(w_qkv, dtype=np.float32)
    b_qkv = np.asarray(b_qkv, dtype=np.float32)
    w_out = np.asarray(w_out, dtype=np.float32)

    per_group = []
    for g in range(2):
        rows = np.concatenate(
            [
                w_qkv[qkv * DIM + g * FEAT : qkv * DIM + (g + 1) * FEAT]
                for qkv in range(3)
            ],
            axis=0,
        )  # [1152, 768]
        wqkvT_g = np.ascontiguousarray(rows.T).astype(_bf16)  # [768, 1152]
        b_rows = np.concatenate(
            [
                b_qkv[qkv * DIM + g * FEAT : qkv * DIM + (g + 1) * FEAT]
                for qkv in range(3)
            ],
            axis=0,
        )  # [1152]
        bias_g = np.ascontiguousarray(b_rows.reshape(9, 128).T)  # [128, 9]
        woutT_g = _round_to_f32r(w_out[:, g * FEAT : (g + 1) * FEAT].T)
        per_group.append((wqkvT_g, bias_g, woutT_g))

    xT_bf = [np.ascontiguousarray(x[b].T).astype(_bf16) for b in range(B)]
    in_maps = []
    for c in range(NCORES):
        b, g = c // 2, c % 2
        wqkvT_g, bias_g, woutT_g = per_group[g]
        in_maps.append(
            {
                "xT": xT_bf[b],
                "wqkvT": wqkvT_g,
                "bqkv": bias_g,
                "woutT": woutT_g,
            }
        )
    return in_maps


def assemble_output(results, b_out):
    """Host-side unshard: sum partials per batch pair, transpose, add bias."""
    b_out = np.asarray(b_out, dtype=np.float32)
    out = np.empty((B, N, DIM), dtype=np.float32)
    for b in range(B):
        pT = results[2 * b]["outT"] + results[2 * b + 1]["outT"]  # [768, 2048]
        out[b] = pT.T + b_out[None, :]
    return out


def kernel(x, w_qkv, b_qkv, w_out, b_out):
    from concourse.bass_utils import run_bass_kernel_spmd

    nc = _get_program()
    in_maps = make_core_inputs(x, w_qkv, b_qkv, w_out)
    res = run_bass_kernel_spmd(nc, in_maps, list(range(NCORES)))
    return assemble_output(res.results, b_out)


# revision 14
# speedup vs baseline: 1.0920x; 1.0278x over previous
"""Multi-head attention forward on 8 Trainium2 NeuronCores (Bass/Tile).

Problem: x[4, 2048, 768] -> qkv proj (w_qkv[2304, 768]) -> 12-head attention
(softmax((q k^T) * 768^-0.5)) -> out proj (w_out[768, 768]).

Sharding: core c handles batch b = c//2 and a group of 6 heads g = c%2
(tensor parallel over heads within a batch pair). Each core computes a
partial output (its heads' contribution through the row-sliced out
projection, transposed: [768, 2048]); the host sums the two partials per
batch, transposes back and adds b_out.

Device-side layout notes (everything transposed so the contraction dim sits
on SBUF partitions):
  xT   [768, 2048]  transposed on the HOST, plain DMA load (no PE transposes)
  qkvT [feat, 2048] = wT.T @ xT via bf16 matmuls
  qk_t [128, 4, N]: head A of the pair in rows 0:64 of chunks 0 (q) / 1 (k),
  head B in rows 64:128 of chunks 2/3; unused halves zeroed so the K=128
  scores contraction sees zero padding.
  scoresT[keys, q]  = kT_tile.T @ qT  (so attn@v needs no transpose)
  softmax without max-subtraction (scores are O(1); exp is safe in fp32);
  denominator comes free from an appended ones-column in v ("v_aug").
  Normalize: reciprocal_approx_fast on DVE, bf16 PE broadcast of the recip
  (deferred into the next block so it never stalls PE), DVE multiply.

Schedule: span-major blocks of 8 half-slots (2 score matmuls + 1 exp each);
attn@v trails its exp by LAG half-slots; qkv chains / v transposes for the
next head pair and out-proj tiles drain from a debt-paced filler queue so
the PE never idles while the ACT engine chews exps.
"""

import os
import sys

import ml_dtypes
import numpy as np

_bf16 = ml_dtypes.bfloat16

if "/opt/trn_rl_repo" not in sys.path:
    sys.path.insert(0, "/opt/trn_rl_repo")

B = 4
N = 2048
DIM = 768
HEADS = 12
DHEAD = 64
SCALE = DIM ** (-0.5)
NCORES = 8
HPC = 6  # heads per core
FEAT = HPC * DHEAD  # 384 per-core attention features

_PROGRAM = None  # (nc,) cached compiled bass program


def _build_program():
    from contextlib import ExitStack

    import concourse.bass as bass
    import concourse.tile as tile
    from concourse import bacc, mybir
    from concourse.masks import make_identity

    f32 = mybir.dt.float32
    f32r = mybir.dt.float32r
    bf16 = mybir.dt.bfloat16
    Alu = mybir.AluOpType
    ActF = mybir.ActivationFunctionType

    nc = bacc.Bacc("TRN2", target_bir_lowering=False, debug=False)

    x_in = nc.dram_tensor("xT", [DIM, N], bf16, kind="ExternalInput")
    wqkvT = nc.dram_tensor("wqkvT", [DIM, 3 * FEAT], bf16, kind="ExternalInput")
    bqkv = nc.dram_tensor("bqkv", [128, 9], f32, kind="ExternalInput")
    woutT = nc.dram_tensor("woutT", [FEAT, DIM], f32r, kind="ExternalInput")
    out_T = nc.dram_tensor("outT", [DIM, N], f32, kind="ExternalOutput")

    NT = N // 128  # 16 key tiles
    KC = DIM // 128  # 6 contraction chunks for dim
    NSPAN = N // 512  # 4 moving spans

    HALF_BUDGET = 300.0  # ns of filler debt accrued per half-slot
    DEBT_CAP = 6000.0
    CHAIN_COST = 2600.0
    VTRANS_COST = 900.0
    VAUGI_COST = 1100.0
    OUTPROJ_COST = 750.0
    ZPAD_COST = 100.0  # gpsimd memsets, nearly free for the PE
    LAG = 2  # attnv trails its scores by this many half-slots

    with tile.TileContext(nc) as tc, ExitStack() as ctx:
        const = ctx.enter_context(tc.tile_pool(name="const", bufs=1))
        identity_bf = const.tile([128, 128], bf16)
        make_identity(nc, identity_bf)
        ones_f32 = const.tile([128, 1], f32)
        nc.vector.memset(ones_f32[:, :], 1.0)
        ones_bf = const.tile([65, 64], bf16)
        nc.vector.tensor_copy(
            out=ones_bf[:, :], in_=ones_f32[0:65, :].to_broadcast((65, 64))
        )
        bias_sb = const.tile([128, 9], f32)
        nc.gpsimd.dma_start(bias_sb[:, :], bqkv[:, :])

        # ---- xT: transposed on host, plain DMA ----
        xt_pool = ctx.enter_context(tc.tile_pool(name="xT", bufs=1))
        xT = xt_pool.tile([128, KC, N], bf16)
        for j in range(KC):
            eng = nc.gpsimd if j % 2 == 0 else nc.sync
            eng.dma_start(xT[:, j, :], x_in[j * 128 : (j + 1) * 128, :])

        wpool = ctx.enter_context(tc.tile_pool(name="w", bufs=1))
        w_all = wpool.tile([128, KC, 3 * FEAT], bf16)
        for j in range(KC):
            eng = nc.gpsimd if j % 2 == 0 else nc.sync
            eng.dma_start(w_all[:, j, :], wqkvT[j * 128 : (j + 1) * 128, :])
        wout_sb = wpool.tile([128, 3, DIM], f32r)
        for c in range(3):
            nc.sync.dma_start(wout_sb[:, c, :], woutT[c * 128 : (c + 1) * 128, :])

        # PSUM: spool 3x[128,2,512] (6 banks) + opool 2x[128,512] (2 banks)
        spool = ctx.enter_context(tc.tile_pool(name="spsum", bufs=3, space="PSUM"))
        opool = ctx.enter_context(tc.tile_pool(name="opsum", bufs=2, space="PSUM"))

        qk_pool = ctx.enter_context(tc.tile_pool(name="qk", bufs=2))
        vt_pool = ctx.enter_context(tc.tile_pool(name="vt", bufs=2))
        vaug_pool = ctx.enter_context(tc.tile_pool(name="vaug", bufs=2))
        exp_pool = ctx.enter_context(tc.tile_pool(name="expT", bufs=6))
        rcpf_pool = ctx.enter_context(tc.tile_pool(name="rcpf", bufs=2))
        rcpb_pool = ctx.enter_context(tc.tile_pool(name="rcpb", bufs=2))
        sbo_pool = ctx.enter_context(tc.tile_pool(name="sbo", bufs=3))
        hst_pool = ctx.enter_context(tc.tile_pool(name="hstage", bufs=2))
        ao_pool = ctx.enter_context(tc.tile_pool(name="attnout", bufs=1))
        attn_outT = ao_pool.tile([128, 3, N], f32r)
        ost_pool = ctx.enter_context(tc.tile_pool(name="ostage", bufs=3))

        def emit_qkv_chain(qk_t, vT_t, idx, m, sp2):
            """One (m, span-pair): 12 matmuls + biased copyback."""
            ps = spool.tile([128, 2, 512], f32, tag="s", name="ps_qkv")
            for u in range(2):
                span = 2 * sp2 + u
                for j in range(KC):
                    nc.tensor.matmul(
                        ps[:, u, :],
                        w_all[:, j, m * 128 : (m + 1) * 128],
                        xT[:, j, span * 512 : (span + 1) * 512],
                        start=(j == 0),
                        stop=(j == KC - 1),
                    )
            cols = slice(sp2 * 1024, (sp2 + 1) * 1024)
            if idx < 2:
                # head A -> chunk idx rows 0:64; head B -> chunk idx+2 rows 64:128
                nc.vector.tensor_scalar(
                    qk_t[0:64, idx, cols].rearrange("p (a b) -> p a b", a=2),
                    ps[0:64, :, :],
                    bias_sb[0:64, m : m + 1],
                    None,
                    Alu.add,
                )
                nc.vector.tensor_scalar(
                    qk_t[64:128, idx + 2, cols].rearrange("p (a b) -> p a b", a=2),
                    ps[64:128, :, :],
                    bias_sb[64:128, m : m + 1],
                    None,
                    Alu.add,
                )
            else:
                nc.vector.tensor_scalar(
                    vT_t[:, cols].rearrange("p (a b) -> p a b", a=2),
                    ps[:, :, :],
                    bias_sb[:, m : m + 1],
                    None,
                    Alu.add,
                )

        def emit_zero_pads(qk_t, vaug_t):
            # zero the unused halves so K/M padding contributes nothing
            nc.gpsimd.memset(qk_t[64:128, 0:2, :], 0.0)
            nc.gpsimd.memset(qk_t[0:64, 2:4, :], 0.0)
            nc.gpsimd.memset(vaug_t[:, :, :], 0.0)

        def emit_vtrans(vaug_t, vT_t, kc0, kcn):
            for kc in range(kc0, kcn):
                tp = spool.tile([128, 128], bf16, tag="s", name="tp_v")
                nc.tensor.transpose(
                    tp[:, :], vT_t[:, kc * 128 : (kc + 1) * 128], identity_bf[:, :]
                )
                nc.vector.tensor_copy(
                    out=vaug_t[:, kc, :].rearrange("p (t c) -> p t c", t=2)[
                        :, :, 0:64
                    ],
                    in_=tp[:, :].rearrange("p (t c) -> p t c", t=2),
                )

        def emit_vaug_init(vaug_t, vT_t):
            ones_cols = vaug_t[:, :, :].rearrange("p k (t c) -> p k t c", t=2)[
                :, :, :, 64:65
            ]
            nc.vector.tensor_copy(
                out=ones_cols, in_=ones_f32[:, :].to_broadcast((128, NT, 2, 1))
            )
            emit_vtrans(vaug_t, vT_t, 0, 4)

        def emit_outproj(m, span):
            """One out-proj tile [128, 512]: 3 matmuls + copyback + DMA."""
            ps = spool.tile([128, 2, 512], f32, tag="s", name="ps_op")
            for c in range(3):
                nc.tensor.matmul(
                    ps[:, 0, :],
                    wout_sb[:, c, m * 128 : (m + 1) * 128],
                    attn_outT[:, c, span * 512 : (span + 1) * 512],
                    start=(c == 0),
                    stop=(c == 2),
                )
            ostage = ost_pool.tile([128, 512], f32, name="ostage", tag="ostage")
            nc.vector.tensor_copy(out=ostage[:, :], in_=ps[:, 0, :])
            nc.gpsimd.dma_start(
                out_T[m * 128 : (m + 1) * 128, span * 512 : (span + 1) * 512],
                ostage[:, :],
            )

        # ---- filler machinery: (cost, closure) FIFO drained by debt ----
        filler_q = []
        debt = [2000.0]
        markers = set()  # emission-order guarantees (see pump_until)

        def pump(amount):
            debt[0] = min(debt[0] + amount, DEBT_CAP)
            while filler_q and debt[0] >= filler_q[0][0]:
                c, fn = filler_q.pop(0)
                fn()
                debt[0] -= c

        def pump_until(marker):
            """Force-drain fillers until `marker` has been emitted. Readers
            of filler-written tiles MUST be emitted after the writer."""
            while marker not in markers and filler_q:
                filler_q.pop(0)[1]()
            assert marker in markers, f"filler marker {marker} never queued"

        def marked(marker, fn):
            def go():
                fn()
                markers.add(marker)

            return go

        def make_pair_tiles():
            qk_t = qk_pool.tile([128, 4, N], bf16, name="qk", tag="qk")
            vT_t = vt_pool.tile([128, N], bf16, name="vT", tag="vT")
            vaug_t = vaug_pool.tile([128, NT, 256], bf16, name="vaug", tag="vaug")
            return qk_t, vT_t, vaug_t

        def push_pair_fillers(hp, qk_t, vT_t, vaug_t):
            """Queue pair hp's prep work as fillers for the prior pair."""
            filler_q.append((ZPAD_COST, lambda: emit_zero_pads(qk_t, vaug_t)))
            for idx, m, sp2 in (
                (0, hp, 0),
                (1, 3 + hp, 0),
                (1, 3 + hp, 1),
                (2, 6 + hp, 0),
                (2, 6 + hp, 1),
            ):
                filler_q.append(
                    (
                        CHAIN_COST,
                        lambda i=idx, mm=m, s=sp2: emit_qkv_chain(qk_t, vT_t, i, mm, s),
                    )
                )
            filler_q.append(
                (
                    CHAIN_COST,
                    marked(
                        ("q1", hp),
                        lambda: emit_qkv_chain(qk_t, vT_t, 0, hp, 1),
                    ),
                )
            )
            filler_q.append((VAUGI_COST, lambda: emit_vaug_init(vaug_t, vT_t)))
            for kc0 in (4, 8, 12):
                fn = lambda k=kc0: emit_vtrans(vaug_t, vT_t, k, k + 4)
                if kc0 == 12:
                    fn = marked(("vtall", hp), fn)
                filler_q.append((VTRANS_COST, fn))

        # ---- attention block machinery ----
        pending = []  # closures popped with LAG half-slots of delay

        def mk_attnv(po, vaug_t, j, et, half):
            def go():
                for u in range(2):
                    kc = 2 * half + u
                    nc.tensor.matmul(
                        po[:, :],
                        vaug_t[:, kc, j * 128 : (j + 1) * 128],
                        et[:, u, :],
                        start=(kc == 0),
                        stop=(kc == 15),
                    )

            return go

        def emit_block(hp, j, span, qk_t, vaug_t, carry_flush, lag=LAG):
            po = opool.tile([128, 512], f32, tag="o", name="po")
            qT = qk_t[:, 2 * j, span * 512 : (span + 1) * 512]
            for half in range(8):
                ps = spool.tile([128, 2, 512], f32, tag="s", name="ps_s")
                for u in range(2):
                    kc = 2 * half + u
                    nc.tensor.matmul(
                        ps[:, u, :],
                        qk_t[:, 2 * j + 1, kc * 128 : (kc + 1) * 128],
                        qT,
                        start=True,
                        stop=True,
                    )
                et = exp_pool.tile([128, 2, 512], bf16)
                nc.scalar.activation(
                    et[:, :, :], ps[:, :, :], ActF.Exp, scale=float(SCALE)
                )
                pending.append(mk_attnv(po, vaug_t, j, et, half))
                if half == 6 and carry_flush is not None:
                    pending.append(carry_flush)
                    carry_flush = None
                pump(HALF_BUDGET)
                while len(pending) > lag:
                    pending.pop(0)()
            # post-block DVE chain: snapshot numerators+denominator, recip
            sb_o = sbo_pool.tile([65, 512], f32, name="sb_o")
            rcp_f = rcpf_pool.tile([65, 512], f32, name="rcp_f")
            rcp_b = rcpb_pool.tile([65, 512], bf16, name="rcp_b")

            def post_dve(po=po, sb_o=sb_o, rcp_f=rcp_f, rcp_b=rcp_b):
                # NOTE: reciprocal_approx_fast mis-executes on HW (garbage
                # out, sim models it fine) — use the exact DVE reciprocal;
                # its ~3.3us is hidden by the deferred flush. Read po's
                # denominator row directly so it needn't wait on the copy.
                nc.vector.reciprocal(rcp_f[64:65, :], po[64:65, :])
                nc.vector.tensor_copy(out=sb_o[:, :], in_=po[0:65, :])
                nc.vector.tensor_copy(out=rcp_b[64:65, :], in_=rcp_f[64:65, :])

            pending.append(post_dve)

            def flush(po=po, sb_o=sb_o, rcp_b=rcp_b, hp=hp, j=j, span=span):
                # PE-broadcast of the recip back into po's own bank
                # (WAR-ordered after the snapshot copy), then multiply.
                nc.tensor.matmul(
                    po[0:64, :],
                    ones_bf[64:65, 0:64],
                    rcp_b[64:65, :],
                    start=True,
                    stop=True,
                )
                if j == 0:
                    ddst = attn_outT[0:64, hp, span * 512 : (span + 1) * 512]
                else:
                    ddst = hst_pool.tile([64, 512], f32r, name="hstage", tag="hstage")
                nc.vector.tensor_tensor(
                    out=ddst, in0=sb_o[0:64, :], in1=po[0:64, :], op=Alu.mult
                )
                if j == 1:
                    nc.gpsimd.dma_start(
                        attn_outT[64:128, hp, span * 512 : (span + 1) * 512],
                        ddst[:, :],
                    )
                    if hp == 2:
                        for m in range(DIM // 128):
                            filler_q.append(
                                (OUTPROJ_COST, lambda mm=m, s=span: emit_outproj(mm, s))
                            )

            return flush

        # ---- prologue: the minimum for the first scores block: q sp0 +
        # both k chains. v/vaug/q-sp1 drain as fillers during block 0,
        # whose attnv+normalize are deferred wholesale into block 1.
        qk0, vT0, vaug0 = make_pair_tiles()
        emit_zero_pads(qk0, vaug0)
        emit_qkv_chain(qk0, vT0, 0, 0, 0)  # q sp0
        emit_qkv_chain(qk0, vT0, 1, 3, 0)  # k sp0
        emit_qkv_chain(qk0, vT0, 1, 3, 1)  # k sp1
        filler_q.append(
            (CHAIN_COST, lambda: emit_qkv_chain(qk0, vT0, 2, 6, 0))  # v sp0
        )
        filler_q.append(
            (CHAIN_COST, lambda: emit_qkv_chain(qk0, vT0, 2, 6, 1))  # v sp1
        )
        filler_q.append(
            (
                CHAIN_COST,
                marked(("q1", 0), lambda: emit_qkv_chain(qk0, vT0, 0, 0, 1)),
            )
        )
        filler_q.append((VAUGI_COST, lambda: emit_vaug_init(vaug0, vT0)))
        for kc0 in (4, 8, 12):
            fn = lambda k=kc0: emit_vtrans(vaug0, vT0, k, k + 4)
            if kc0 == 12:
                fn = marked(("vtall", 0), fn)
            filler_q.append((VTRANS_COST, fn))

        # ---- attention: 3 pairs x 4 spans x 2 heads, span-major ----
        cur = (qk0, vT0, vaug0)
        carry = None
        first = True
        for hp in range(3):
            qk_t, vT_t, vaug_t = cur
            if hp < 2:
                nxt = make_pair_tiles()
                push_pair_fillers(hp + 1, *nxt)
                cur = nxt
            for span in range(NSPAN):
                for j in range(2):
                    if first:
                        # defer ALL of block 0's attnv into block 1: its
                        # vaug is still being filled by fillers
                        carry = emit_block(hp, j, span, qk_t, vaug_t, carry, lag=99)
                        first = False
                        continue
                    if span == 0 and j == 0 and hp > 0:
                        pump_until(("vtall", hp))  # emission-order guard
                    if hp == 0 and span == 0 and j == 1:
                        pump_until(("vtall", 0))
                    if span == 2 and j == 0:
                        pump_until(("q1", hp))
                    carry = emit_block(hp, j, span, qk_t, vaug_t, carry)
        while pending:
            pending.pop(0)()
        carry()
        while filler_q:
            filler_q.pop(0)[1]()

    nc.compile()
    return nc


def _get_program():
    global _PROGRAM
    if _PROGRAM is None:
        _PROGRAM = _build_program()
    return _PROGRAM


def _round_to_f32r(a):
    """Round fp32 to the PE's fp32r format: 11-bit mantissa, low 12 bits zero
    (round to nearest, ties away handled approximately via +0x7FF + lsb)."""
    u = np.ascontiguousarray(a, dtype=np.float32).view(np.uint32)
    r = u + np.uint32(0x7FF) + ((u >> np.uint32(12)) & np.uint32(1))
    r &= np.uint32(0xFFFFF000)
    return r.view(np.float32)


def make_core_inputs(x, w_qkv, b_qkv, w_out):
    """Host-side shard: per-core input dicts for cores 0..7."""
    x = np.asarray(x, dtype=np.float32)
    w_qkv = np.asarray(w_qkv, dtype=np.float32)
    b_qkv = np.asarray(b_qkv, dtype=np.float32)
    w_out = np.asarray(w_out, dtype=np.float32)

    per_group = []
    for g in range(2):
        rows = np.concatenate(
            [
                w_qkv[qkv * DIM + g * FEAT : qkv * DIM + (g + 1) * FEAT]
                for qkv in range(3)
            ],
            axis=0,
        )  # [1152, 768]
        wqkvT_g = np.ascontiguousarray(rows.T).astype(_bf16)  # [768, 1152]
        b_rows = np.concatenate(
            [
                b_qkv[qkv * DIM + g * FEAT : qkv * DIM + (g + 1) * FEAT]
                for qkv in range(3)
            ],
            axis=0,
        )  # [1152]
        bias_g = np.ascontiguousarray(b_rows.reshape(9, 128).T)  # [128, 9]
        woutT_g = _round_to_f32r(w_out[:, g * FEAT : (g + 1) * FEAT].T)
        per_group.append((wqkvT_g, bias_g, woutT_g))

    xT_bf = [np.ascontiguousarray(x[b].T).astype(_bf16) for b in range(B)]
    in_maps = []
    for c in range(NCORES):
        b, g = c // 2, c % 2
        wqkvT_g, bias_g, woutT_g = per_group[g]
        in_maps.append(
            {
                "xT": xT_bf[b],
                "wqkvT": wqkvT_g,
                "bqkv": bias_g,
                "woutT": woutT_g,
            }
        )
    return in_maps


def assemble_output(results, b_out):
    """Host-side unshard: sum partials per batch pair, transpose, add bias."""
    b_out = np.asarray(b_out, dtype=np.float32)
    out = np.empty((B, N, DIM), dtype=np.float32)
    for b in range(B):
        pT = results[2 * b]["outT"] + results[2 * b + 1]["outT"]  # [768, 2048]
        out[b] = pT.T + b_out[None, :]
    return out


def kernel(x, w_qkv, b_qkv, w_out, b_out):
    from concourse.bass_utils import run_bass_kernel_spmd

    nc = _get_program()
    in_maps = make_core_inputs(x, w_qkv, b_qkv, w_out)
    res = run_bass_kernel_spmd(nc, in_maps, list(range(NCORES)))
    return assemble_output(res.results, b_out)


# revision 22
# speedup vs baseline: 1.2678x; 1.1610x over previous
"""Multi-head attention forward on 8 Trainium2 NeuronCores (Bass/Tile).

Problem: x[4, 2048, 768] -> qkv proj (w_qkv[2304, 768]) -> 12-head attention
(softmax((q k^T) * 768^-0.5)) -> out proj (w_out[768, 768]).

Sharding: core c handles batch b = c//2 and a group of 6 heads g = c%2
(tensor parallel over heads within a batch pair). Each core computes a
partial output (its heads' contribution through the row-sliced out
projection, transposed: [768, 2048]); the host sums the two partials per
batch, transposes back and adds b_out.

Device-side layout notes (everything transposed so the contraction dim sits
on SBUF partitions):
  xT   [768, 2048]  transposed on the HOST, plain DMA load (no PE transposes)
  qkvT [feat, 2048] = wT.T @ xT via bf16 matmuls
  qk_t [128, 4, N]: head A of the pair in rows 0:64 of chunks 0 (q) / 1 (k),
  head B in rows 64:128 of chunks 2/3; unused halves zeroed so the K=128
  scores contraction sees zero padding.
  scoresT[keys, q]  = kT_tile.T @ qT  (so attn@v needs no transpose)
  softmax without max-subtraction (scores are O(1); exp is safe in fp32);
  denominator comes free from an appended ones-column in v ("v_aug").
  Normalize: reciprocal_approx_fast on DVE, bf16 PE broadcast of the recip
  (deferred into the next block so it never stalls PE), DVE multiply.

Schedule: span-major blocks of 8 half-slots (2 score matmuls + 1 exp each);
attn@v trails its exp by LAG half-slots; qkv chains / v transposes for the
next head pair and out-proj tiles drain from a debt-paced filler queue so
the PE never idles while the ACT engine chews exps.
"""

import os
import sys

import ml_dtypes
import numpy as np

_bf16 = ml_dtypes.bfloat16

if "/opt/trn_rl_repo" not in sys.path:
    sys.path.insert(0, "/opt/trn_rl_repo")

B = 4
N = 2048
DIM = 768
HEADS = 12
DHEAD = 64
SCALE = DIM ** (-0.5)
NCORES = 8
HPC = 6  # heads per core
FEAT = HPC * DHEAD  # 384 per-core attention features

_PROGRAM = None  # (nc,) cached compiled bass program


def _build_program():
    from contextlib import ExitStack

    import concourse.bass as bass
    import concourse.tile as tile
    from concourse import bacc, mybir
    from concourse.masks import make_identity

    f32 = mybir.dt.float32
    f32r = mybir.dt.float32r
    bf16 = mybir.dt.bfloat16
    Alu = mybir.AluOpType
    ActF = mybir.ActivationFunctionType

    nc = bacc.Bacc("TRN2", target_bir_lowering=False, debug=False)

    x_in = nc.dram_tensor("xT", [DIM, N], bf16, kind="ExternalInput")
    wqkvT = nc.dram_tensor("wqkvT", [DIM, 3 * FEAT], bf16, kind="ExternalInput")
    bqkv = nc.dram_tensor("bqkv", [128, 9], f32, kind="ExternalInput")
    woutT = nc.dram_tensor("woutT", [FEAT, DIM], f32r, kind="ExternalInput")
    out_T = nc.dram_tensor("outT", [DIM, N], f32, kind="ExternalOutput")

    NT = N // 128  # 16 key tiles
    KC = DIM // 128  # 6 contraction chunks for dim
    NSPAN = N // 512  # 4 moving spans

    HALF_BUDGET = 300.0  # ns of filler debt accrued per half-slot
    DEBT_CAP = 6000.0
    CHAIN_COST = 2600.0
    VTRANS_COST = 900.0
    VAUGI_COST = 1100.0
    OUTPROJ_COST = 750.0
    ZPAD_COST = 100.0  # gpsimd memsets, nearly free for the PE
    LAG = 2  # attnv trails its scores by this many half-slots

    with tile.TileContext(nc) as tc, ExitStack() as ctx:
        const = ctx.enter_context(tc.tile_pool(name="const", bufs=1))
        identity_bf = const.tile([128, 128], bf16)
        make_identity(nc, identity_bf)
        ones_f32 = const.tile([128, 1], f32)
        nc.vector.memset(ones_f32[:, :], 1.0)
        ones_bf = const.tile([65, 64], bf16)
        nc.vector.tensor_copy(
            out=ones_bf[:, :], in_=ones_f32[0:65, :].to_broadcast((65, 64))
        )
        bias_sb = const.tile([128, 9], f32)
        nc.gpsimd.dma_start(bias_sb[:, :], bqkv[:, :])

        # ---- xT: transposed on host, plain DMA. Interleave xT/w chunks
        # across both queues so chain matmul j has xT[j]+w[j] early.
        xt_pool = ctx.enter_context(tc.tile_pool(name="xT", bufs=1))
        xT = xt_pool.tile([128, KC, N], bf16)
        wpool = ctx.enter_context(tc.tile_pool(name="w", bufs=1))
        w_all = wpool.tile([128, KC, 3 * FEAT], bf16)
        for j in range(KC):
            eng_x = nc.gpsimd if j % 2 == 0 else nc.sync
            eng_w = nc.sync if j % 2 == 0 else nc.gpsimd
            eng_x.dma_start(xT[:, j, :], x_in[j * 128 : (j + 1) * 128, :])
            eng_w.dma_start(w_all[:, j, :], wqkvT[j * 128 : (j + 1) * 128, :])
        wout_sb = wpool.tile([128, 3, DIM], f32r)
        for c in range(3):
            nc.sync.dma_start(wout_sb[:, c, :], woutT[c * 128 : (c + 1) * 128, :])

        # PSUM: spool 3x[128,2,512] (6 banks) + opool 2x[128,512] (2 banks)
        spool = ctx.enter_context(tc.tile_pool(name="spsum", bufs=3, space="PSUM"))
        opool = ctx.enter_context(tc.tile_pool(name="opsum", bufs=2, space="PSUM"))

        qk_pool = ctx.enter_context(tc.tile_pool(name="qk", bufs=2))
        vt_pool = ctx.enter_context(tc.tile_pool(name="vt", bufs=2))
        vaug_pool = ctx.enter_context(tc.tile_pool(name="vaug", bufs=2))
        exp_pool = ctx.enter_context(tc.tile_pool(name="expT", bufs=6))
        den8_pool = ctx.enter_context(tc.tile_pool(name="den8", bufs=2))
        rcp8_pool = ctx.enter_context(tc.tile_pool(name="rcp8", bufs=2))
        rcp8b_pool = ctx.enter_context(tc.tile_pool(name="rcp8b", bufs=2))
        rrow_pool = ctx.enter_context(tc.tile_pool(name="rrow", bufs=2))
        sbo_pool = ctx.enter_context(tc.tile_pool(name="sbo", bufs=3))
        hst_pool = ctx.enter_context(tc.tile_pool(name="hstage", bufs=2))
        ao_pool = ctx.enter_context(tc.tile_pool(name="attnout", bufs=1))
        attn_outT = ao_pool.tile([128, 3, N], f32r)
        ost_pool = ctx.enter_context(tc.tile_pool(name="ostage", bufs=3))

        def emit_qkv_chain(qk_t, vT_t, idx, m, sp2):
            """One (m, span-pair): 12 matmuls + biased copyback."""
            ps = spool.tile([128, 2, 512], f32, tag="s", name="ps_qkv")
            for u in range(2):
                span = 2 * sp2 + u
                for j in range(KC):
                    nc.tensor.matmul(
                        ps[:, u, :],
                        w_all[:, j, m * 128 : (m + 1) * 128],
                        xT[:, j, span * 512 : (span + 1) * 512],
                        start=(j == 0),
                        stop=(j == KC - 1),
                    )
            cols = slice(sp2 * 1024, (sp2 + 1) * 1024)
            if idx < 2:
                # head A -> chunk idx rows 0:64; head B -> chunk idx+2 rows 64:128
                nc.vector.tensor_scalar(
                    qk_t[0:64, idx, cols].rearrange("p (a b) -> p a b", a=2),
                    ps[0:64, :, :],
                    bias_sb[0:64, m : m + 1],
                    None,
                    Alu.add,
                )
                nc.vector.tensor_scalar(
                    qk_t[64:128, idx + 2, cols].rearrange("p (a b) -> p a b", a=2),
                    ps[64:128, :, :],
                    bias_sb[64:128, m : m + 1],
                    None,
                    Alu.add,
                )
            else:
                nc.vector.tensor_scalar(
                    vT_t[:, cols].rearrange("p (a b) -> p a b", a=2),
                    ps[:, :, :],
                    bias_sb[:, m : m + 1],
                    None,
                    Alu.add,
                )

        def emit_zero_pads_qk(qk_t):
            # zero the unused halves so K padding contributes nothing.
            # Emitted ONCE per physical buffer (DVE, idle during DMA wait);
            # chain copybacks never touch the pad rows, so zeros persist
            # across the 2-buffer pair rotation.
            nc.vector.memset(qk_t[64:128, 0:2, :], 0.0)
            nc.vector.memset(qk_t[0:64, 2:4, :], 0.0)

        def emit_zero_pads_vaug(vaug_t):
            # once per buffer: zero all of vaug, then set the ones columns;
            # vtrans only rewrites data cols 0:64 of each half, so pad zeros
            # and ones survive pair rotation.
            nc.gpsimd.memset(vaug_t[:, :, :], 0.0)
            ones_cols = vaug_t[:, :, :].rearrange("p k (t c) -> p k t c", t=2)[
                :, :, :, 64:65
            ]
            nc.vector.tensor_copy(
                out=ones_cols, in_=ones_f32[:, :].to_broadcast((128, NT, 2, 1))
            )

        def emit_vtrans(vaug_t, vT_t, kc0, kcn):
            for kc in range(kc0, kcn):
                tp = spool.tile([128, 128], bf16, tag="s", name="tp_v")
                nc.tensor.transpose(
                    tp[:, :], vT_t[:, kc * 128 : (kc + 1) * 128], identity_bf[:, :]
                )
                nc.vector.tensor_copy(
                    out=vaug_t[:, kc, :].rearrange("p (t c) -> p t c", t=2)[
                        :, :, 0:64
                    ],
                    in_=tp[:, :].rearrange("p (t c) -> p t c", t=2),
                )

        def emit_vaug_init(vaug_t, vT_t):
            emit_vtrans(vaug_t, vT_t, 0, 4)

        def emit_outproj(m, span):
            """One out-proj tile [128, 512]: 3 matmuls + copyback + DMA."""
            ps = spool.tile([128, 2, 512], f32, tag="s", name="ps_op")
            for c in range(3):
                nc.tensor.matmul(
                    ps[:, 0, :],
                    wout_sb[:, c, m * 128 : (m + 1) * 128],
                    attn_outT[:, c, span * 512 : (span + 1) * 512],
                    start=(c == 0),
                    stop=(c == 2),
                )
            ostage = ost_pool.tile([128, 512], f32, name="ostage", tag="ostage")
            nc.vector.tensor_copy(out=ostage[:, :], in_=ps[:, 0, :])
            nc.gpsimd.dma_start(
                out_T[m * 128 : (m + 1) * 128, span * 512 : (span + 1) * 512],
                ostage[:, :],
            )

        # ---- filler machinery: (cost, closure) FIFO drained by debt ----
        filler_q = []
        debt = [2000.0]
        markers = set()  # emission-order guarantees (see pump_until)

        def pump(amount):
            debt[0] = min(debt[0] + amount, DEBT_CAP)
            while filler_q and debt[0] >= filler_q[0][0]:
                c, fn = filler_q.pop(0)
                fn()
                debt[0] -= c

        def pump_until(marker):
            """Force-drain fillers until `marker` has been emitted. Readers
            of filler-written tiles MUST be emitted after the writer."""
            while marker not in markers and filler_q:
                filler_q.pop(0)[1]()
            assert marker in markers, f"filler marker {marker} never queued"

        def marked(marker, fn):
            def go():
                fn()
                markers.add(marker)

            return go

        def make_pair_tiles():
            qk_t = qk_pool.tile([128, 4, N], bf16, name="qk", tag="qk")
            vT_t = vt_pool.tile([128, N], bf16, name="vT", tag="vT")
            vaug_t = vaug_pool.tile([128, NT, 256], bf16, name="vaug", tag="vaug")
            return qk_t, vT_t, vaug_t

        def push_pair_fillers(hp, qk_t, vT_t, vaug_t):
            """Queue pair hp's prep work as fillers for the prior pair."""
            for idx, m, sp2 in (
                (0, hp, 0),
                (1, 3 + hp, 0),
                (1, 3 + hp, 1),
                (2, 6 + hp, 0),
                (2, 6 + hp, 1),
            ):
                filler_q.append(
                    (
                        CHAIN_COST,
                        lambda i=idx, mm=m, s=sp2: emit_qkv_chain(qk_t, vT_t, i, mm, s),
                    )
                )
            filler_q.append(
                (
                    CHAIN_COST,
                    marked(
                        ("q1", hp),
                        lambda: emit_qkv_chain(qk_t, vT_t, 0, hp, 1),
                    ),
                )
            )
            filler_q.append((VAUGI_COST, lambda: emit_vaug_init(vaug_t, vT_t)))
            for kc0 in (4, 8, 12):
                fn = lambda k=kc0: emit_vtrans(vaug_t, vT_t, k, k + 4)
                if kc0 == 12:
                    fn = marked(("vtall", hp), fn)
                filler_q.append((VTRANS_COST, fn))

        # ---- attention block machinery ----
        pending = []  # closures popped with LAG half-slots of delay

        def mk_attnv(po, vaug_t, j, et, half):
            def go():
                for u in range(2):
                    kc = 2 * half + u
                    nc.tensor.matmul(
                        po[:, :],
                        vaug_t[:, kc, j * 128 : (j + 1) * 128],
                        et[:, u, :],
                        start=(kc == 0),
                        stop=(kc == 15),
                    )

            return go

        def emit_block(hp, j, span, qk_t, vaug_t, carry_flush, lag=LAG):
            po = opool.tile([128, 512], f32, tag="o", name="po")
            qT = qk_t[:, 2 * j, span * 512 : (span + 1) * 512]
            for half in range(8):
                ps = spool.tile([128, 2, 512], f32, tag="s", name="ps_s")
                for u in range(2):
                    kc = 2 * half + u
                    nc.tensor.matmul(
                        ps[:, u, :],
                        qk_t[:, 2 * j + 1, kc * 128 : (kc + 1) * 128],
                        qT,
                        start=True,
                        stop=True,
                    )
                et = exp_pool.tile([128, 2, 512], bf16)
                nc.scalar.activation(
                    et[:, :, :], ps[:, :, :], ActF.Exp, scale=float(SCALE)
                )
                pending.append(mk_attnv(po, vaug_t, j, et, half))
                if half == 6 and carry_flush is not None:
                    pending.append(carry_flush)
                    carry_flush = None
                pump(HALF_BUDGET)
                while len(pending) > lag:
                    pending.pop(0)()
            # post-block: snapshot numerators+denominator; reciprocal is
            # lane-spread: scatter the 512 denominators over 64 partitions
            # via SBUF DMA so the DVE reciprocal runs 64-wide (~0.2us, not
            # 3.3us — the serial [1,512] version plugged the DVE queue and
            # stalled spool rotation behind it).
            sb_o = sbo_pool.tile([65, 512], f32, name="sb_o")
            den8 = den8_pool.tile([64, 8], f32, name="den8")
            rcp8 = rcp8_pool.tile([64, 8], f32, name="rcp8")
            rcp8b = rcp8b_pool.tile([64, 8], bf16, name="rcp8b")
            rcp_row = rrow_pool.tile([65, 512], bf16, name="rcp_row")

            def post_dve(po=po, sb_o=sb_o, den8=den8, rcp8=rcp8, rcp8b=rcp8b,
                         rcp_row=rcp_row):
                nc.vector.tensor_copy(out=sb_o[:, :], in_=po[0:65, :])
                nc.sync.dma_start(den8[:, :], sb_o[64:65, :])
                nc.vector.reciprocal(rcp8[:, :], den8[:, :])
                nc.vector.tensor_copy(out=rcp8b[:, :], in_=rcp8[:, :])
                nc.sync.dma_start(rcp_row[64:65, :], rcp8b[:, :])

            pending.append(post_dve)

            def flush(po=po, sb_o=sb_o, rcp_row=rcp_row, hp=hp, j=j, span=span):
                # PE-broadcast of the recip back into po's own bank
                # (WAR-ordered after the snapshot copy), then multiply.
                nc.tensor.matmul(
                    po[0:64, :],
                    ones_bf[64:65, 0:64],
                    rcp_row[64:65, :],
                    start=True,
                    stop=True,
                )
                if j == 0:
                    ddst = attn_outT[0:64, hp, span * 512 : (span + 1) * 512]
                else:
                    ddst = hst_pool.tile([64, 512], f32r, name="hstage", tag="hstage")
                nc.vector.tensor_tensor(
                    out=ddst, in0=sb_o[0:64, :], in1=po[0:64, :], op=Alu.mult
                )
                if j == 1:
                    nc.gpsimd.dma_start(
                        attn_outT[64:128, hp, span * 512 : (span + 1) * 512],
                        ddst[:, :],
                    )
                    if hp == 2:
                        for m in range(DIM // 128):
                            filler_q.append(
                                (OUTPROJ_COST, lambda mm=m, s=span: emit_outproj(mm, s))
                            )

            return flush

        # ---- prologue: the minimum for the first scores block: q sp0 +
        # both k chains. v/vaug/q-sp1 drain as fillers during block 0,
        # whose attnv+normalize are deferred wholesale into block 1.
        # Both physical buffer sets are allocated up front and their pads
        # zeroed exactly once (they persist across pair rotation).
        tilesets = [make_pair_tiles(), make_pair_tiles()]
        qk0, vT0, vaug0 = tilesets[0]
        emit_zero_pads_qk(tilesets[0][0])
        emit_zero_pads_qk(tilesets[1][0])
        emit_zero_pads_vaug(tilesets[0][2])
        emit_zero_pads_vaug(tilesets[1][2])
        emit_qkv_chain(qk0, vT0, 0, 0, 0)  # q sp0
        emit_qkv_chain(qk0, vT0, 1, 3, 0)  # k sp0
        emit_qkv_chain(qk0, vT0, 1, 3, 1)  # k sp1
        filler_q.append(
            (CHAIN_COST, lambda: emit_qkv_chain(qk0, vT0, 2, 6, 0))  # v sp0
        )
        filler_q.append(
            (CHAIN_COST, lambda: emit_qkv_chain(qk0, vT0, 2, 6, 1))  # v sp1
        )
        filler_q.append(
            (
                CHAIN_COST,
                marked(("q1", 0), lambda: emit_qkv_chain(qk0, vT0, 0, 0, 1)),
            )
        )
        filler_q.append((VAUGI_COST, lambda: emit_vaug_init(vaug0, vT0)))
        for kc0 in (4, 8, 12):
            fn = lambda k=kc0: emit_vtrans(vaug0, vT0, k, k + 4)
            if kc0 == 12:
                fn = marked(("vtall", 0), fn)
            filler_q.append((VTRANS_COST, fn))

        # ---- attention: 3 pairs x 4 spans x 2 heads, span-major ----
        carry = None
        first = True
        for hp in range(3):
            qk_t, vT_t, vaug_t = tilesets[hp % 2]
            if hp < 2:
                push_pair_fillers(hp + 1, *tilesets[(hp + 1) % 2])
            for span in range(NSPAN):
                for j in range(2):
                    if first:
                        # defer ALL of block 0's attnv into block 1: its
                        # vaug is still being filled by fillers
                        carry = emit_block(hp, j, span, qk_t, vaug_t, carry, lag=99)
                        first = False
                        continue
                    if span == 0 and j == 0 and hp > 0:
                        pump_until(("vtall", hp))  # emission-order guard
                    if hp == 0 and span == 0 and j == 1:
                        pump_until(("vtall", 0))
                    if span == 2 and j == 0:
                        pump_until(("q1", hp))
                    carry = emit_block(hp, j, span, qk_t, vaug_t, carry)
        while pending:
            pending.pop(0)()
        carry()
        while filler_q:
            filler_q.pop(0)[1]()

    nc.compile()
    return nc


def _get_program():
    global _PROGRAM
    if _PROGRAM is None:
        _PROGRAM = _build_program()
    return _PROGRAM


def _round_to_f32r(a):
    """Round fp32 to the PE's fp32r format: 11-bit mantissa, low 12 bits zero
    (round to nearest, ties away handled approximately via +0x7FF + lsb)."""
    u = np.ascontiguousarray(a, dtype=np.float32).view(np.uint32)
    r = u + np.uint32(0x7FF) + ((u >> np.uint32(12)) & np.uint32(1))
    r &= np.uint32(0xFFFFF000)
    return r.view(np.float32)


def make_core_inputs(x, w_qkv, b_qkv, w_out):
    """Host-side shard: per-core input dicts for cores 0..7."""
    x = np.asarray(x, dtype=np.float32)
    w_qkv = np.asarray(w_qkv, dtype=np.float32)
    b_qkv = np.asarray(b_qkv, dtype=np.float32)
    w_out = np.asarray(w_out, dtype=np.float32)

    per_group = []
    for g in range(2):
        rows = np.concatenate(
            [
                w_qkv[qkv * DIM + g * FEAT : qkv * DIM + (g + 1) * FEAT]
                for qkv in range(3)
            ],
            axis=0,
        )  # [1152, 768]
        wqkvT_g = np.ascontiguousarray(rows.T).astype(_bf16)  # [768, 1152]
        b_rows = np.concatenate(
            [
                b_qkv[qkv * DIM + g * FEAT : qkv * DIM + (g + 1) * FEAT]
                for qkv in range(3)
            ],
            axis=0,
        )  # [1152]
        bias_g = np.ascontiguousarray(b_rows.reshape(9, 128).T)  # [128, 9]
        woutT_g = _round_to_f32r(w_out[:, g * FEAT : (g + 1) * FEAT].T)
        per_group.append((wqkvT_g, bias_g, woutT_g))

    xT_bf = [np.ascontiguousarray(x[b].T).astype(_bf16) for b in range(B)]
    in_maps = []
    for c in range(NCORES):
        b, g = c // 2, c % 2
        wqkvT_g, bias_g, woutT_g = per_group[g]
        in_maps.append(
            {
                "xT": xT_bf[b],
                "wqkvT": wqkvT_g,
                "bqkv": bias_g,
                "woutT": woutT_g,
            }
        )
    return in_maps


def assemble_output(results, b_out):
    """Host-side unshard: sum partials per batch pair, transpose, add bias."""
    b_out = np.asarray(b_out, dtype=np.float32)
    out = np.empty((B, N, DIM), dtype=np.float32)
    for b in range(B):
        pT = results[2 * b]["outT"] + results[2 * b + 1]["outT"]  # [768, 2048]
        out[b] = pT.T + b_out[None, :]
    return out


def kernel(x, w_qkv, b_qkv, w_out, b_out):
    from concourse.bass_utils import run_bass_kernel_spmd

    nc = _get_program()
    in_maps = make_core_inputs(x, w_qkv, b_qkv, w_out)
    res = run_bass_kernel_spmd(nc, in_maps, list(range(NCORES)))
    return assemble_output(res.results, b_out)


# revision 27
# speedup vs baseline: 1.2915x; 1.0187x over previous
"""Multi-head attention forward on 8 Trainium2 NeuronCores (Bass/Tile).

Problem: x[4, 2048, 768] -> qkv proj (w_qkv[2304, 768]) -> 12-head attention
(softmax((q k^T) * 768^-0.5)) -> out proj (w_out[768, 768]).

Sharding: core c handles batch b = c//2 and a group of 6 heads g = c%2
(tensor parallel over heads within a batch pair). Each core computes a
partial output (its heads' contribution through the row-sliced out
projection, transposed: [768, 2048]); the host sums the two partials per
batch, transposes back and adds b_out.

Device-side layout notes (everything transposed so the contraction dim sits
on SBUF partitions):
  xT   [768, 2048]  transposed on the HOST, plain DMA load (no PE transposes)
  qkvT [feat, 2048] = wT.T @ xT via bf16 matmuls
  qk_t [128, 4, N]: head A of the pair in rows 0:64 of chunks 0 (q) / 1 (k),
  head B in rows 64:128 of chunks 2/3; unused halves zeroed so the K=128
  scores contraction sees zero padding.
  scoresT[keys, q]  = kT_tile.T @ qT  (so attn@v needs no transpose)
  softmax without max-subtraction (scores are O(1); exp is safe in fp32);
  denominator comes free from an appended ones-column in v ("v_aug").
  Normalize: reciprocal_approx_fast on DVE, bf16 PE broadcast of the recip
  (deferred into the next block so it never stalls PE), DVE multiply.

Schedule: span-major blocks of 8 half-slots (2 score matmuls + 1 exp each);
attn@v trails its exp by LAG half-slots; qkv chains / v transposes for the
next head pair and out-proj tiles drain from a debt-paced filler queue so
the PE never idles while the ACT engine chews exps.
"""

import os
import sys

import ml_dtypes
import numpy as np

_bf16 = ml_dtypes.bfloat16

if "/opt/trn_rl_repo" not in sys.path:
    sys.path.insert(0, "/opt/trn_rl_repo")

B = 4
N = 2048
DIM = 768
HEADS = 12
DHEAD = 64
SCALE = DIM ** (-0.5)
NCORES = 8
HPC = 6  # heads per core
FEAT = HPC * DHEAD  # 384 per-core attention features

_PROGRAM = None  # (nc,) cached compiled bass program


def _build_program():
    from contextlib import ExitStack

    import concourse.bass as bass
    import concourse.tile as tile
    from concourse import bacc, mybir
    from concourse.masks import make_identity

    f32 = mybir.dt.float32
    f32r = mybir.dt.float32r
    bf16 = mybir.dt.bfloat16
    Alu = mybir.AluOpType
    ActF = mybir.ActivationFunctionType

    nc = bacc.Bacc("TRN2", target_bir_lowering=False, debug=False)

    x_in = nc.dram_tensor("xT", [DIM, N], bf16, kind="ExternalInput")
    wqkvT = nc.dram_tensor("wqkvT", [DIM, 3 * FEAT], bf16, kind="ExternalInput")
    bqkv = nc.dram_tensor("bqkv", [128, 9], f32, kind="ExternalInput")
    woutT = nc.dram_tensor("woutT", [FEAT, DIM], f32r, kind="ExternalInput")
    out_T = nc.dram_tensor("outT", [DIM, N], f32, kind="ExternalOutput")

    NT = N // 128  # 16 key tiles
    KC = DIM // 128  # 6 contraction chunks for dim
    NSPAN = N // 512  # 4 moving spans

    HALF_BUDGET = 300.0  # ns of filler debt accrued per half-slot
    DEBT_CAP = 6000.0
    CHAIN_COST = 2600.0
    VTRANS_COST = 900.0
    VAUGI_COST = 1100.0
    OUTPROJ_COST = 750.0
    LAG = 3  # attnv trails its scores by this many half-slots

    with tile.TileContext(nc) as tc, ExitStack() as ctx:
        const = ctx.enter_context(tc.tile_pool(name="const", bufs=1))
        identity_bf = const.tile([128, 128], bf16)
        make_identity(nc, identity_bf)
        ones_f32 = const.tile([128, 1], f32)
        nc.vector.memset(ones_f32[:, :], 1.0)
        ones_bf = const.tile([65, 64], bf16)
        nc.vector.tensor_copy(
            out=ones_bf[:, :], in_=ones_f32[0:65, :].to_broadcast((65, 64))
        )
        bias_sb = const.tile([128, 9], f32)
        nc.gpsimd.dma_start(bias_sb[:, :], bqkv[:, :])

        # ---- xT: transposed on host, plain DMA. Interleave xT/w chunks
        # across both queues so chain matmul j has xT[j]+w[j] early.
        xt_pool = ctx.enter_context(tc.tile_pool(name="xT", bufs=1))
        xT = xt_pool.tile([128, KC, N], bf16)
        wpool = ctx.enter_context(tc.tile_pool(name="w", bufs=1))
        w_all = wpool.tile([128, KC, 3 * FEAT], bf16)
        # xT split per (chunk, span) so chain matmul (j, span) only waits
        # its own 128KB slice; weights on the other queue in j order.
        for j in range(KC):
            nc.sync.dma_start(w_all[:, j, :], wqkvT[j * 128 : (j + 1) * 128, :])
            for s in range(NSPAN):
                nc.gpsimd.dma_start(
                    xT[:, j, s * 512 : (s + 1) * 512],
                    x_in[j * 128 : (j + 1) * 128, s * 512 : (s + 1) * 512],
                )
        wout_sb = wpool.tile([128, 3, DIM], f32r)
        for c in range(3):
            nc.sync.dma_start(wout_sb[:, c, :], woutT[c * 128 : (c + 1) * 128, :])

        # PSUM: spool 3x[128,2,512] (6 banks) + opool 2x[128,512] (2 banks)
        spool = ctx.enter_context(tc.tile_pool(name="spsum", bufs=3, space="PSUM"))
        opool = ctx.enter_context(tc.tile_pool(name="opsum", bufs=2, space="PSUM"))

        qk_pool = ctx.enter_context(tc.tile_pool(name="qk", bufs=2))
        vt_pool = ctx.enter_context(tc.tile_pool(name="vt", bufs=2))
        vaug_pool = ctx.enter_context(tc.tile_pool(name="vaug", bufs=2))
        exp_pool = ctx.enter_context(tc.tile_pool(name="expT", bufs=6))
        den8_pool = ctx.enter_context(tc.tile_pool(name="den8", bufs=2))
        rcp8_pool = ctx.enter_context(tc.tile_pool(name="rcp8", bufs=2))
        rcp8b_pool = ctx.enter_context(tc.tile_pool(name="rcp8b", bufs=2))
        rrow_pool = ctx.enter_context(tc.tile_pool(name="rrow", bufs=2))
        sbo_pool = ctx.enter_context(tc.tile_pool(name="sbo", bufs=3))
        hst_pool = ctx.enter_context(tc.tile_pool(name="hstage", bufs=2))
        ao_pool = ctx.enter_context(tc.tile_pool(name="attnout", bufs=1))
        attn_outT = ao_pool.tile([128, 3, N], f32r)
        ost_pool = ctx.enter_context(tc.tile_pool(name="ostage", bufs=3))

        def emit_qkv_chain(qk_t, vT_t, idx, m, sp2):
            """One (m, span-pair): 12 matmuls + biased copyback."""
            ps = spool.tile([128, 2, 512], f32, tag="s", name="ps_qkv")
            for u in range(2):
                span = 2 * sp2 + u
                for j in range(KC):
                    nc.tensor.matmul(
                        ps[:, u, :],
                        w_all[:, j, m * 128 : (m + 1) * 128],
                        xT[:, j, span * 512 : (span + 1) * 512],
                        start=(j == 0),
                        stop=(j == KC - 1),
                    )
            cols = slice(sp2 * 1024, (sp2 + 1) * 1024)
            if idx < 2:
                # head A -> chunk idx rows 0:64; head B -> chunk idx+2 rows 64:128
                nc.vector.tensor_scalar(
                    qk_t[0:64, idx, cols].rearrange("p (a b) -> p a b", a=2),
                    ps[0:64, :, :],
                    bias_sb[0:64, m : m + 1],
                    None,
                    Alu.add,
                )
                nc.vector.tensor_scalar(
                    qk_t[64:128, idx + 2, cols].rearrange("p (a b) -> p a b", a=2),
                    ps[64:128, :, :],
                    bias_sb[64:128, m : m + 1],
                    None,
                    Alu.add,
                )
            else:
                nc.vector.tensor_scalar(
                    vT_t[:, cols].rearrange("p (a b) -> p a b", a=2),
                    ps[:, :, :],
                    bias_sb[:, m : m + 1],
                    None,
                    Alu.add,
                )

        def emit_zero_pads_qk(qk_t):
            # zero the unused halves so K padding contributes nothing.
            # Emitted ONCE per physical buffer (DVE, idle during DMA wait);
            # chain copybacks never touch the pad rows, so zeros persist
            # across the 2-buffer pair rotation.
            nc.vector.memset(qk_t[64:128, 0:2, :], 0.0)
            nc.vector.memset(qk_t[0:64, 2:4, :], 0.0)

        def emit_zero_pads_vaug(vaug_t):
            # once per buffer: zero all of vaug, then set the ones columns;
            # vtrans only rewrites data cols 0:64 of each half, so pad zeros
            # and ones survive pair rotation.
            nc.gpsimd.memset(vaug_t[:, :, :], 0.0)
            ones_cols = vaug_t[:, :, :].rearrange("p k (t c) -> p k t c", t=2)[
                :, :, :, 64:65
            ]
            nc.vector.tensor_copy(
                out=ones_cols, in_=ones_f32[:, :].to_broadcast((128, NT, 2, 1))
            )

        def emit_vtrans(vaug_t, vT_t, kc0, kcn):
            for kc in range(kc0, kcn):
                tp = spool.tile([128, 128], bf16, tag="s", name="tp_v")
                nc.tensor.transpose(
                    tp[:, :], vT_t[:, kc * 128 : (kc + 1) * 128], identity_bf[:, :]
                )
                nc.vector.tensor_copy(
                    out=vaug_t[:, kc, :].rearrange("p (t c) -> p t c", t=2)[
                        :, :, 0:64
                    ],
                    in_=tp[:, :].rearrange("p (t c) -> p t c", t=2),
                )

        def emit_vaug_init(vaug_t, vT_t):
            emit_vtrans(vaug_t, vT_t, 0, 4)

        def emit_outproj(m, span):
            """One out-proj tile [128, 512]: 3 matmuls + copyback + DMA."""
            ps = spool.tile([128, 2, 512], f32, tag="s", name="ps_op")
            for c in range(3):
                nc.tensor.matmul(
                    ps[:, 0, :],
                    wout_sb[:, c, m * 128 : (m + 1) * 128],
                    attn_outT[:, c, span * 512 : (span + 1) * 512],
                    start=(c == 0),
                    stop=(c == 2),
                )
            ostage = ost_pool.tile([128, 512], f32, name="ostage", tag="ostage")
            nc.vector.tensor_copy(out=ostage[:, :], in_=ps[:, 0, :])
            nc.gpsimd.dma_start(
                out_T[m * 128 : (m + 1) * 128, span * 512 : (span + 1) * 512],
                ostage[:, :],
            )

        # ---- filler machinery: (cost, closure) FIFO drained by debt ----
        filler_q = []
        debt = [2000.0]
        markers = set()  # emission-order guarantees (see pump_until)

        def pump(amount):
            debt[0] = min(debt[0] + amount, DEBT_CAP)
            while filler_q and debt[0] >= filler_q[0][0]:
                c, fn = filler_q.pop(0)
                fn()
                debt[0] -= c

        def pump_until(marker):
            """Force-drain fillers until `marker` has been emitted. Readers
            of filler-written tiles MUST be emitted after the writer."""
            while marker not in markers and filler_q:
                filler_q.pop(0)[1]()
            assert marker in markers, f"filler marker {marker} never queued"

        def marked(marker, fn):
            def go():
                fn()
                markers.add(marker)

            return go

        def make_pair_tiles():
            qk_t = qk_pool.tile([128, 4, N], bf16, name="qk", tag="qk")
            vT_t = vt_pool.tile([128, N], bf16, name="vT", tag="vT")
            vaug_t = vaug_pool.tile([128, NT, 256], bf16, name="vaug", tag="vaug")
            return qk_t, vT_t, vaug_t

        def push_pair_fillers(hp, qk_t, vT_t, vaug_t):
            """Queue pair hp's prep work as fillers for the prior pair."""
            for idx, m, sp2 in (
                (0, hp, 0),
                (1, 3 + hp, 0),
                (1, 3 + hp, 1),
                (2, 6 + hp, 0),
                (2, 6 + hp, 1),
            ):
                filler_q.append(
                    (
                        CHAIN_COST,
                        lambda i=idx, mm=m, s=sp2: emit_qkv_chain(qk_t, vT_t, i, mm, s),
                    )
                )
            filler_q.append(
                (
                    CHAIN_COST,
                    marked(
                        ("q1", hp),
                        lambda: emit_qkv_chain(qk_t, vT_t, 0, hp, 1),
                    ),
                )
            )
            filler_q.append((VAUGI_COST, lambda: emit_vaug_init(vaug_t, vT_t)))
            for kc0 in (4, 8, 12):
                fn = lambda k=kc0: emit_vtrans(vaug_t, vT_t, k, k + 4)
                if kc0 == 12:
                    fn = marked(("vtall", hp), fn)
                filler_q.append((VTRANS_COST, fn))

        # ---- attention block machinery ----
        pending = []  # closures popped with LAG half-slots of delay
        flush_done = {}  # hp=2 spans: both flushes seen -> outproj ready

        def mk_attnv(po, vaug_t, j, et, half):
            def go():
                for u in range(2):
                    kc = 2 * half + u
                    nc.tensor.matmul(
                        po[:, :],
                        vaug_t[:, kc, j * 128 : (j + 1) * 128],
                        et[:, u, :],
                        start=(kc == 0),
                        stop=(kc == 15),
                    )

            return go

        def emit_block(hp, j, span, qk_t, vaug_t, carry_flush, lag=LAG):
            po = opool.tile([128, 512], f32, tag="o", name="po")
            qT = qk_t[:, 2 * j, span * 512 : (span + 1) * 512]
            for half in range(8):
                ps = spool.tile([128, 2, 512], f32, tag="s", name="ps_s")
                for u in range(2):
                    kc = 2 * half + u
                    nc.tensor.matmul(
                        ps[:, u, :],
                        qk_t[:, 2 * j + 1, kc * 128 : (kc + 1) * 128],
                        qT,
                        start=True,
                        stop=True,
                    )
                et = exp_pool.tile([128, 2, 512], bf16)
                nc.scalar.activation(
                    et[:, :, :], ps[:, :, :], ActF.Exp, scale=float(SCALE)
                )
                pending.append(mk_attnv(po, vaug_t, j, et, half))
                if half == 6 and carry_flush is not None:
                    pending.append(carry_flush)
                    carry_flush = None
                pump(HALF_BUDGET)
                while len(pending) > lag:
                    pending.pop(0)()
            # post-block: snapshot numerators+denominator; reciprocal is
            # lane-spread: scatter the 512 denominators over 64 partitions
            # via SBUF DMA so the DVE reciprocal runs 64-wide (~0.2us, not
            # 3.3us — the serial [1,512] version plugged the DVE queue and
            # stalled spool rotation behind it).
            sb_o = sbo_pool.tile([65, 512], f32, name="sb_o")
            den8 = den8_pool.tile([64, 8], f32, name="den8")
            rcp8 = rcp8_pool.tile([64, 8], f32, name="rcp8")
            rcp8b = rcp8b_pool.tile([64, 8], bf16, name="rcp8b")
            rcp_row = rrow_pool.tile([65, 512], bf16, name="rcp_row")

            def post_dve(po=po, sb_o=sb_o, den8=den8, rcp8=rcp8, rcp8b=rcp8b,
                         rcp_row=rcp_row):
                nc.vector.tensor_copy(out=sb_o[:, :], in_=po[0:65, :])
                nc.sync.dma_start(den8[:, :], sb_o[64:65, :])
                nc.vector.reciprocal(rcp8[:, :], den8[:, :])
                nc.vector.tensor_copy(out=rcp8b[:, :], in_=rcp8[:, :])
                nc.sync.dma_start(rcp_row[64:65, :], rcp8b[:, :])

            pending.append(post_dve)

            def flush(po=po, sb_o=sb_o, rcp_row=rcp_row, hp=hp, j=j, span=span):
                # PE-broadcast of the recip back into po's own bank
                # (WAR-ordered after the snapshot copy), then multiply.
                nc.tensor.matmul(
                    po[0:64, :],
                    ones_bf[64:65, 0:64],
                    rcp_row[64:65, :],
                    start=True,
                    stop=True,
                )
                if j == 0:
                    ddst = attn_outT[0:64, hp, span * 512 : (span + 1) * 512]
                else:
                    ddst = hst_pool.tile([64, 512], f32r, name="hstage", tag="hstage")
                nc.vector.tensor_tensor(
                    out=ddst, in0=sb_o[0:64, :], in1=po[0:64, :], op=Alu.mult
                )
                if j == 1:
                    nc.sync.dma_start(
                        attn_outT[64:128, hp, span * 512 : (span + 1) * 512],
                        ddst[:, :],
                    )
                if hp == 2:
                    flush_done[span] = flush_done.get(span, 0) + 1
                    if flush_done[span] == 2:
                        for m in range(DIM // 128):
                            filler_q.append(
                                (OUTPROJ_COST, lambda mm=m, s=span: emit_outproj(mm, s))
                            )

            return flush

        # ---- prologue: the minimum for the first scores block: q sp0 +
        # both k chains. v/vaug/q-sp1 drain as fillers during block 0,
        # whose attnv+normalize are deferred wholesale into block 1.
        # Both physical buffer sets are allocated up front and their pads
        # zeroed exactly once (they persist across pair rotation).
        tilesets = [make_pair_tiles(), make_pair_tiles()]
        qk0, vT0, vaug0 = tilesets[0]
        emit_zero_pads_qk(tilesets[0][0])
        emit_zero_pads_qk(tilesets[1][0])
        emit_zero_pads_vaug(tilesets[0][2])
        emit_zero_pads_vaug(tilesets[1][2])
        emit_qkv_chain(qk0, vT0, 0, 0, 0)  # q sp0
        emit_qkv_chain(qk0, vT0, 1, 3, 0)  # k sp0
        emit_qkv_chain(qk0, vT0, 1, 3, 1)  # k sp1
        filler_q.append(
            (CHAIN_COST, lambda: emit_qkv_chain(qk0, vT0, 2, 6, 0))  # v sp0
        )
        filler_q.append(
            (CHAIN_COST, lambda: emit_qkv_chain(qk0, vT0, 2, 6, 1))  # v sp1
        )
        filler_q.append(
            (
                CHAIN_COST,
                marked(("q1", 0), lambda: emit_qkv_chain(qk0, vT0, 0, 0, 1)),
            )
        )
        filler_q.append((VAUGI_COST, lambda: emit_vaug_init(vaug0, vT0)))
        for kc0 in (4, 8, 12):
            fn = lambda k=kc0: emit_vtrans(vaug0, vT0, k, k + 4)
            if kc0 == 12:
                fn = marked(("vtall", 0), fn)
            filler_q.append((VTRANS_COST, fn))

        # ---- attention: 3 pairs x 4 spans x 2 heads, span-major ----
        carry = None
        first = True
        for hp in range(3):
            qk_t, vT_t, vaug_t = tilesets[hp % 2]
            if hp < 2:
                push_pair_fillers(hp + 1, *tilesets[(hp + 1) % 2])
            for span in range(NSPAN):
                # last span of the last pair runs j=1 first so its hstage
                # DMA overlaps the j=0 block, shortening the serial tail
                js = (1, 0) if (hp == 2 and span == 3) else (0, 1)
                for j in js:
                    if first:
                        # defer ALL of block 0's attnv into block 1: its
                        # vaug is still being filled by fillers
                        carry = emit_block(hp, j, span, qk_t, vaug_t, carry, lag=99)
                        first = False
                        continue
                    if span == 0 and j == 0 and hp > 0:
                        pump_until(("vtall", hp))  # emission-order guard
                    if hp == 0 and span == 0 and j == 1:
                        pump_until(("vtall", 0))
                    if span == 2 and j == 0:
                        pump_until(("q1", hp))
                    carry = emit_block(hp, j, span, qk_t, vaug_t, carry)
        while pending:
            pending.pop(0)()
        carry()
        while filler_q:
            filler_q.pop(0)[1]()

    nc.compile()
    return nc


def _get_program():
    global _PROGRAM
    if _PROGRAM is None:
        _PROGRAM = _build_program()
    return _PROGRAM


def _round_to_f32r(a):
    """Round fp32 to the PE's fp32r format: 11-bit mantissa, low 12 bits zero
    (round to nearest, ties away handled approximately via +0x7FF + lsb)."""
    u = np.ascontiguousarray(a, dtype=np.float32).view(np.uint32)
    r = u + np.uint32(0x7FF) + ((u >> np.uint32(12)) & np.uint32(1))
    r &= np.uint32(0xFFFFF000)
    return r.view(np.float32)


def make_core_inputs(x, w_qkv, b_qkv, w_out):
    """Host-side shard: per-core input dicts for cores 0..7."""
    x = np.asarray(x, dtype=np.float32)
    w_qkv = np.asarray(w_qkv, dtype=np.float32)
    b_qkv = np.asarray(b_qkv, dtype=np.float32)
    w_out = np.asarray(w_out, dtype=np.float32)

    per_group = []
    for g in range(2):
        rows = np.concatenate(
            [
                w_qkv[qkv * DIM + g * FEAT : qkv * DIM + (g + 1) * FEAT]
                for qkv in range(3)
            ],
            axis=0,
        )  # [1152, 768]
        wqkvT_g = np.ascontiguousarray(rows.T).astype(_bf16)  # [768, 1152]
        b_rows = np.concatenate(
            [
                b_qkv[qkv * DIM + g * FEAT : qkv * DIM + (g + 1) * FEAT]
                for qkv in range(3)
            ],
            axis=0,
        )  # [1152]
        bias_g = np.ascontiguousarray(b_rows.reshape(9, 128).T)  # [128, 9]
        woutT_g = _round_to_f32r(w_out[:, g * FEAT : (g + 1) * FEAT].T)
        per_group.append((wqkvT_g, bias_g, woutT_g))

    xT_bf = [np.ascontiguousarray(x[b].T).astype(_bf16) for b in range(B)]
    in_maps = []
    for c in range(NCORES):
        b, g = c // 2, c % 2
        wqkvT_g, bias_g, woutT_g = per_group[g]
        in_maps.append(
            {
                "xT": xT_bf[b],
                "wqkvT": wqkvT_g,
                "bqkv": bias_g,
                "woutT": woutT_g,
            }
        )
    return in_maps


def assemble_output(results, b_out):
    """Host-side unshard: sum partials per batch pair, transpose, add bias."""
    b_out = np.asarray(b_out, dtype=np.float32)
    out = np.empty((B, N, DIM), dtype=np.float32)
    for b in range(B):
        pT = results[2 * b]["outT"] + results[2 * b + 1]["outT"]  # [768, 2048]
        out[b] = pT.T + b_out[None, :]
    return out


def kernel(x, w_qkv, b_qkv, w_out, b_out):
    from concourse.bass_utils import run_bass_kernel_spmd

    nc = _get_program()
    in_maps = make_core_inputs(x, w_qkv, b_qkv, w_out)
    res = run_bass_kernel_spmd(nc, in_maps, list(range(NCORES)))
    return assemble_output(res.results, b_out)


# revision 40
# speedup vs baseline: 1.3863x; 1.0734x over previous
"""Multi-head attention forward on 8 Trainium2 NeuronCores (Bass/Tile).

Problem: x[4, 2048, 768] -> qkv proj (w_qkv[2304, 768]) -> 12-head attention
(softmax((q k^T) * 768^-0.5)) -> out proj (w_out[768, 768]).

Sharding: core c handles batch b = c//2 and a group of 6 heads g = c%2
(tensor parallel over heads within a batch pair). Each core computes a
partial output (its heads' contribution through the row-sliced out
projection, transposed: [768, 2048]); the host sums the two partials per
batch, transposes back and adds b_out.

Device-side layout notes (everything transposed so the contraction dim sits
on SBUF partitions):
  xT   [768, 2048]  transposed on the HOST, plain DMA load (no PE transposes)
  qkvT [feat, 2048] = wT.T @ xT via bf16 matmuls
  qk_t [128, 4, N]: head A of the pair in rows 0:64 of chunks 0 (q) / 1 (k),
  head B in rows 64:128 of chunks 2/3; unused halves zeroed so the K=128
  scores contraction sees zero padding.
  scoresT[keys, q]  = kT_tile.T @ qT  (so attn@v needs no transpose)
  softmax without max-subtraction (scores are O(1); exp is safe in fp32);
  denominator comes free from an appended ones-column in v ("v_aug").
  Normalize: reciprocal_approx_fast on DVE, bf16 PE broadcast of the recip
  (deferred into the next block so it never stalls PE), DVE multiply.

Schedule: span-major blocks of 8 half-slots (2 score matmuls + 1 exp each);
attn@v trails its exp by LAG half-slots; qkv chains / v transposes for the
next head pair and out-proj tiles drain from a debt-paced filler queue so
the PE never idles while the ACT engine chews exps.
"""

import os
import sys

import ml_dtypes
import numpy as np

_bf16 = ml_dtypes.bfloat16

if "/opt/trn_rl_repo" not in sys.path:
    sys.path.insert(0, "/opt/trn_rl_repo")

B = 4
N = 2048
DIM = 768
HEADS = 12
DHEAD = 64
SCALE = DIM ** (-0.5)
NCORES = 8
HPC = 6  # heads per core
FEAT = HPC * DHEAD  # 384 per-core attention features

_PROGRAM = None  # (nc,) cached compiled bass program


def _build_program():
    from contextlib import ExitStack

    import concourse.bass as bass
    import concourse.tile as tile
    from concourse import bacc, mybir
    from concourse.masks import make_identity

    f32 = mybir.dt.float32
    f32r = mybir.dt.float32r
    bf16 = mybir.dt.bfloat16
    Alu = mybir.AluOpType
    ActF = mybir.ActivationFunctionType

    nc = bacc.Bacc("TRN2", target_bir_lowering=False, debug=False)

    x_in = nc.dram_tensor("xT", [DIM, N], bf16, kind="ExternalInput")
    wqkvT = nc.dram_tensor("wqkvT", [DIM, 3 * FEAT], bf16, kind="ExternalInput")
    bqkv = nc.dram_tensor("bqkv", [128, 9], f32, kind="ExternalInput")
    woutT = nc.dram_tensor("woutT", [FEAT, DIM], f32r, kind="ExternalInput")
    out_T = nc.dram_tensor("outT", [DIM, N], f32, kind="ExternalOutput")

    NT = N // 128  # 16 key tiles
    KC = DIM // 128  # 6 contraction chunks for dim
    NSPAN = N // 512  # 4 moving spans

    HALF_BUDGET = 350.0  # ns of filler debt accrued per half-slot
    DEBT_CAP = 3000.0  # bounds the backlog a single slot can release
    CHAIN_COST = 2600.0
    VTRANS_COST = 900.0
    VAUGI_COST = 1100.0
    OUTPROJ_COST = 750.0
    LAG = 3  # attnv trails its scores by this many half-slots

    with tile.TileContext(nc) as tc, ExitStack() as ctx:
        const = ctx.enter_context(tc.tile_pool(name="const", bufs=1))
        identity_bf = const.tile([128, 128], bf16)
        make_identity(nc, identity_bf)
        ones_f32 = const.tile([128, 1], f32)
        nc.vector.memset(ones_f32[:, :], 1.0)
        bias_sb = const.tile([128, 9], f32)
        nc.gpsimd.dma_start(bias_sb[:, :], bqkv[:, :])

        # ---- xT: transposed on host, plain DMA. Interleave xT/w chunks
        # across both queues so chain matmul j has xT[j]+w[j] early.
        xt_pool = ctx.enter_context(tc.tile_pool(name="xT", bufs=1))
        xT = xt_pool.tile([128, KC, N], bf16)
        wpool = ctx.enter_context(tc.tile_pool(name="w", bufs=1))
        w_all = wpool.tile([128, KC, 3 * FEAT], bf16)
        # xT chunks on gpsimd, weights on sync: chain matmul j needs
        # (xT[j], w[j]); issue cost stays low (6 DMAs per queue).
        for j in range(KC):
            nc.sync.dma_start(w_all[:, j, :], wqkvT[j * 128 : (j + 1) * 128, :])
            nc.gpsimd.dma_start(xT[:, j, :], x_in[j * 128 : (j + 1) * 128, :])
        wout_sb = wpool.tile([128, 3, DIM], f32r)
        for c in range(3):
            nc.sync.dma_start(wout_sb[:, c, :], woutT[c * 128 : (c + 1) * 128, :])

        # PSUM: spool 3x[128,2,512] (6 banks) + opool 2x[128,512] (2 banks)
        spool = ctx.enter_context(tc.tile_pool(name="spsum", bufs=3, space="PSUM"))
        opool = ctx.enter_context(tc.tile_pool(name="opsum", bufs=2, space="PSUM"))

        qk_pool = ctx.enter_context(tc.tile_pool(name="qk", bufs=2))
        vt_pool = ctx.enter_context(tc.tile_pool(name="vt", bufs=2))
        vaug_pool = ctx.enter_context(tc.tile_pool(name="vaug", bufs=2))
        # deep: blocks 0-1 defer ALL their attnv pops to block 2+, so up to
        # ~18 exp tiles are simultaneously live at warmup
        exp_pool = ctx.enter_context(tc.tile_pool(name="expT", bufs=20))
        den8_pool = ctx.enter_context(tc.tile_pool(name="den8", bufs=2))
        rcp8_pool = ctx.enter_context(tc.tile_pool(name="rcp8", bufs=2))
        rcp8b_pool = ctx.enter_context(tc.tile_pool(name="rcp8b", bufs=2))
        rrow_pool = ctx.enter_context(tc.tile_pool(name="rrow", bufs=2))
        rbc_pool = ctx.enter_context(tc.tile_pool(name="rbc", bufs=2))
        sbo_pool = ctx.enter_context(tc.tile_pool(name="sbo", bufs=3))
        hst_pool = ctx.enter_context(tc.tile_pool(name="hstage", bufs=2))
        ao_pool = ctx.enter_context(tc.tile_pool(name="attnout", bufs=1))
        attn_outT = ao_pool.tile([128, 3, N], f32r)
        ost_pool = ctx.enter_context(tc.tile_pool(name="ostage", bufs=3))

        def emit_qkv_chain(qk_t, vT_t, idx, m, sp2):
            """One (m, span-pair): 12 matmuls + biased copyback."""
            ps = spool.tile([128, 2, 512], f32, tag="s", name="ps_qkv")
            for u in range(2):
                span = 2 * sp2 + u
                for j in range(KC):
                    nc.tensor.matmul(
                        ps[:, u, :],
                        w_all[:, j, m * 128 : (m + 1) * 128],
                        xT[:, j, span * 512 : (span + 1) * 512],
                        start=(j == 0),
                        stop=(j == KC - 1),
                    )
            cols = slice(sp2 * 1024, (sp2 + 1) * 1024)
            if idx < 2:
                # head A -> chunk idx rows 0:64; head B -> chunk idx+2 rows 64:128
                nc.vector.tensor_scalar(
                    qk_t[0:64, idx, cols].rearrange("p (a b) -> p a b", a=2),
                    ps[0:64, :, :],
                    bias_sb[0:64, m : m + 1],
                    None,
                    Alu.add,
                )
                nc.vector.tensor_scalar(
                    qk_t[64:128, idx + 2, cols].rearrange("p (a b) -> p a b", a=2),
                    ps[64:128, :, :],
                    bias_sb[64:128, m : m + 1],
                    None,
                    Alu.add,
                )
            else:
                nc.vector.tensor_scalar(
                    vT_t[:, cols].rearrange("p (a b) -> p a b", a=2),
                    ps[:, :, :],
                    bias_sb[:, m : m + 1],
                    None,
                    Alu.add,
                )

        def emit_zero_pads_qk(qk_t):
            # zero the unused halves so K padding contributes nothing.
            # Emitted ONCE per physical buffer (DVE, idle during DMA wait);
            # chain copybacks never touch the pad rows, so zeros persist
            # across the 2-buffer pair rotation.
            nc.vector.memset(qk_t[64:128, 0:2, :], 0.0)
            nc.vector.memset(qk_t[0:64, 2:4, :], 0.0)

        def emit_zero_pads_vaug(vaug_t):
            # once per buffer: zero all of vaug, then set the ones columns;
            # vtrans only rewrites data cols 0:64 of each half, so pad zeros
            # and ones survive pair rotation.
            nc.gpsimd.memset(vaug_t[:, :, :], 0.0)
            ones_cols = vaug_t[:, :, :].rearrange("p k (t c) -> p k t c", t=2)[
                :, :, :, 64:65
            ]
            nc.vector.tensor_copy(
                out=ones_cols, in_=ones_f32[:, :].to_broadcast((128, NT, 2, 1))
            )

        def emit_vtrans(vaug_t, vT_t, kc0, kcn):
            for kc in range(kc0, kcn):
                tp = spool.tile([128, 128], bf16, tag="s", name="tp_v")
                nc.tensor.transpose(
                    tp[:, :], vT_t[:, kc * 128 : (kc + 1) * 128], identity_bf[:, :]
                )
                nc.vector.tensor_copy(
                    out=vaug_t[:, kc, :].rearrange("p (t c) -> p t c", t=2)[
                        :, :, 0:64
                    ],
                    in_=tp[:, :].rearrange("p (t c) -> p t c", t=2),
                )

        def emit_vaug_init(vaug_t, vT_t):
            emit_vtrans(vaug_t, vT_t, 0, 4)

        def emit_outproj(m, span):
            """One out-proj tile [128, 512]: 3 matmuls + copyback + DMA."""
            ps = spool.tile([128, 2, 512], f32, tag="s", name="ps_op")
            for c in range(3):
                nc.tensor.matmul(
                    ps[:, 0, :],
                    wout_sb[:, c, m * 128 : (m + 1) * 128],
                    attn_outT[:, c, span * 512 : (span + 1) * 512],
                    start=(c == 0),
                    stop=(c == 2),
                )
            ostage = ost_pool.tile([128, 512], f32, name="ostage", tag="ostage")
            nc.vector.tensor_copy(out=ostage[:, :], in_=ps[:, 0, :])
            nc.gpsimd.dma_start(
                out_T[m * 128 : (m + 1) * 128, span * 512 : (span + 1) * 512],
                ostage[:, :],
            )

        # ---- filler machinery: (cost, closure) FIFO drained by debt ----
        filler_q = []
        debt = [3000.0]
        markers = set()  # emission-order guarantees (see pump_until)

        def pump(amount):
            # at most ONE filler per slot: spreads bursts (e.g. 6 outproj
            # landing at once must not all fire in one slot, starving ACT)
            debt[0] = min(debt[0] + amount, DEBT_CAP)
            if filler_q and debt[0] >= filler_q[0][0]:
                c, fn = filler_q.pop(0)
                fn()
                debt[0] -= c

        def pump_until(marker):
            """Force-drain fillers until `marker` has been emitted. Readers
            of filler-written tiles MUST be emitted after the writer."""
            while marker not in markers and filler_q:
                filler_q.pop(0)[1]()
            assert marker in markers, f"filler marker {marker} never queued"

        def marked(marker, fn):
            def go():
                fn()
                markers.add(marker)

            return go

        def make_pair_tiles():
            qk_t = qk_pool.tile([128, 4, N], bf16, name="qk", tag="qk")
            vT_t = vt_pool.tile([128, N], bf16, name="vT", tag="vT")
            vaug_t = vaug_pool.tile([128, NT, 256], bf16, name="vaug", tag="vaug")
            return qk_t, vT_t, vaug_t

        def push_pair_fillers(hp, qk_t, vT_t, vaug_t):
            """Queue pair hp's prep work as fillers for the prior pair."""
            for idx, m, sp2 in (
                (0, hp, 0),
                (1, 3 + hp, 0),
                (1, 3 + hp, 1),
                (2, 6 + hp, 0),
                (2, 6 + hp, 1),
            ):
                filler_q.append(
                    (
                        CHAIN_COST,
                        lambda i=idx, mm=m, s=sp2: emit_qkv_chain(qk_t, vT_t, i, mm, s),
                    )
                )
            filler_q.append(
                (
                    CHAIN_COST,
                    marked(
                        ("q1", hp),
                        lambda: emit_qkv_chain(qk_t, vT_t, 0, hp, 1),
                    ),
                )
            )
            filler_q.append((VAUGI_COST, lambda: emit_vaug_init(vaug_t, vT_t)))
            for kc0 in (4, 8, 12):
                fn = lambda k=kc0: emit_vtrans(vaug_t, vT_t, k, k + 4)
                if kc0 == 12:
                    fn = marked(("vtall", hp), fn)
                filler_q.append((VTRANS_COST, fn))

        # ---- attention block machinery ----
        pending = []  # closures popped with LAG half-slots of delay
        flush_done = {}  # hp=2 spans: both flushes seen -> outproj ready

        def mk_attnv(po, vaug_t, j, et, half):
            def go():
                for u in range(2):
                    kc = 2 * half + u
                    nc.tensor.matmul(
                        po[:, :],
                        vaug_t[:, kc, j * 128 : (j + 1) * 128],
                        et[:, u, :],
                        start=(kc == 0),
                        stop=(kc == 15),
                    )

            return go

        def emit_block(hp, j, span, qk_t, vaug_t, carry_flush, lag=LAG):
            po = opool.tile([128, 512], f32, tag="o", name="po")
            qT = qk_t[:, 2 * j, span * 512 : (span + 1) * 512]
            for half in range(8):
                ps = spool.tile([128, 2, 512], f32, tag="s", name="ps_s")
                for u in range(2):
                    kc = 2 * half + u
                    nc.tensor.matmul(
                        ps[:, u, :],
                        qk_t[:, 2 * j + 1, kc * 128 : (kc + 1) * 128],
                        qT,
                        start=True,
                        stop=True,
                    )
                et = exp_pool.tile([128, 2, 512], bf16)
                nc.scalar.activation(
                    et[:, :, :], ps[:, :, :], ActF.Exp, scale=float(SCALE)
                )
                pending.append(mk_attnv(po, vaug_t, j, et, half))
                if half == 6 and carry_flush is not None:
                    pending.append(carry_flush)
                    carry_flush = None
                pump(HALF_BUDGET)
                npop = 0
                while len(pending) > lag and npop < 3:
                    pending.pop(0)()
                    npop += 1
            # post-block: snapshot numerators+denominator; reciprocal is
            # lane-spread: scatter the 512 denominators over 64 partitions
            # via SBUF DMA so the DVE reciprocal runs 64-wide (~0.2us, not
            # 3.3us — the serial [1,512] version plugged the DVE queue and
            # stalled spool rotation behind it).
            sb_o = sbo_pool.tile([65, 512], f32, name="sb_o")
            den8 = den8_pool.tile([64, 8], f32, name="den8")
            rcp8 = rcp8_pool.tile([64, 8], f32, name="rcp8")
            rcp8b = rcp8b_pool.tile([64, 8], bf16, name="rcp8b")
            rcp_row = rrow_pool.tile([1, 512], bf16, name="rcp_row")
            rcp_bc = rbc_pool.tile([64, 512], bf16, name="rcp_bc")

            def post_dve(po=po, sb_o=sb_o, den8=den8, rcp8=rcp8, rcp8b=rcp8b,
                         rcp_row=rcp_row, rcp_bc=rcp_bc):
                nc.vector.tensor_copy(out=sb_o[:, :], in_=po[0:65, :])
                nc.sync.dma_start(den8[:, :], sb_o[64:65, :])
                nc.vector.reciprocal(rcp8[:, :], den8[:, :])
                nc.vector.tensor_copy(out=rcp8b[:, :], in_=rcp8[:, :])
                nc.sync.dma_start(rcp_row[0:1, :], rcp8b[:, :])
                # gpsimd partition-broadcast: replicate the recip row to 64
                # partitions (frees the PE of 24 broadcast matmuls and
                # releases po as soon as the snapshot copy is done)
                nc.gpsimd.partition_broadcast(rcp_bc[:, :], rcp_row[0:1, :])

            pending.append(post_dve)

            def flush(sb_o=sb_o, rcp_bc=rcp_bc, hp=hp, j=j, span=span):
                if j == 0:
                    ddst = attn_outT[0:64, hp, span * 512 : (span + 1) * 512]
                else:
                    ddst = hst_pool.tile([64, 512], f32r, name="hstage", tag="hstage")
                nc.vector.tensor_tensor(
                    out=ddst, in0=sb_o[0:64, :], in1=rcp_bc[0:64, :], op=Alu.mult
                )
                if j == 1:
                    nc.sync.dma_start(
                        attn_outT[64:128, hp, span * 512 : (span + 1) * 512],
                        ddst[:, :],
                    )
                if hp == 2:
                    flush_done[span] = flush_done.get(span, 0) + 1
                    if flush_done[span] == 2:
                        for m in range(DIM // 128):
                            filler_q.append(
                                (OUTPROJ_COST, lambda mm=m, s=span: emit_outproj(mm, s))
                            )

            return flush

        # ---- prologue: the minimum for the first scores block: q sp0 +
        # both k chains. v/vaug/q-sp1 drain as fillers during block 0,
        # whose attnv+normalize are deferred wholesale into block 1.
        # Both physical buffer sets are allocated up front and their pads
        # zeroed exactly once (they persist across pair rotation).
        tilesets = [make_pair_tiles(), make_pair_tiles()]
        qk0, vT0, vaug0 = tilesets[0]
        emit_zero_pads_qk(tilesets[0][0])
        emit_zero_pads_qk(tilesets[1][0])
        emit_zero_pads_vaug(tilesets[0][2])
        emit_zero_pads_vaug(tilesets[1][2])
        emit_qkv_chain(qk0, vT0, 0, 0, 0)  # q sp0
        emit_qkv_chain(qk0, vT0, 1, 3, 0)  # k sp0
        emit_qkv_chain(qk0, vT0, 1, 3, 1)  # k sp1
        filler_q.append(
            (CHAIN_COST, lambda: emit_qkv_chain(qk0, vT0, 2, 6, 0))  # v sp0
        )
        filler_q.append(
            (CHAIN_COST, lambda: emit_qkv_chain(qk0, vT0, 2, 6, 1))  # v sp1
        )
        filler_q.append((VAUGI_COST, lambda: emit_vaug_init(vaug0, vT0)))
        for kc0 in (4, 8, 12):
            fn = lambda k=kc0: emit_vtrans(vaug0, vT0, k, k + 4)
            if kc0 == 12:
                fn = marked(("vtall", 0), fn)
            filler_q.append((VTRANS_COST, fn))
        filler_q.append(
            (
                CHAIN_COST,
                marked(("q1", 0), lambda: emit_qkv_chain(qk0, vT0, 0, 0, 1)),
            )
        )

        # ---- attention: 3 pairs x 4 spans x 2 heads, span-major ----
        carry = None
        blk = 0
        for hp in range(3):
            qk_t, vT_t, vaug_t = tilesets[hp % 2]
            if hp < 2:
                push_pair_fillers(hp + 1, *tilesets[(hp + 1) % 2])
            for span in range(NSPAN):
                # last span of the last pair runs j=1 first so its hstage
                # DMA overlaps the j=0 block, shortening the serial tail
                js = (1, 0) if (hp == 2 and span == 3) else (0, 1)
                for j in js:
                    if span == 0 and j == js[0] and hp > 0:
                        pump_until(("vtall", hp))  # emission-order guard
                    if blk == 2:
                        pump_until(("vtall", 0))
                    if span == 2 and j == js[0]:
                        pump_until(("q1", hp))
                    # first two blocks: defer ALL attnv (their vaug is
                    # still being filled by fillers); drain paced later
                    lag = 99 if blk < 2 else LAG
                    carry = emit_block(hp, j, span, qk_t, vaug_t, carry, lag=lag)
                    blk += 1
        while pending:
            pending.pop(0)()
        carry()
        while filler_q:
            filler_q.pop(0)[1]()

    nc.compile()
    return nc


def _get_program():
    global _PROGRAM
    if _PROGRAM is None:
        _PROGRAM = _build_program()
    return _PROGRAM


def _round_to_f32r(a):
    """Round fp32 to the PE's fp32r format: 11-bit mantissa, low 12 bits zero
    (round to nearest, ties away handled approximately via +0x7FF + lsb)."""
    u = np.ascontiguousarray(a, dtype=np.float32).view(np.uint32)
    r = u + np.uint32(0x7FF) + ((u >> np.uint32(12)) & np.uint32(1))
    r &= np.uint32(0xFFFFF000)
    return r.view(np.float32)


def make_core_inputs(x, w_qkv, b_qkv, w_out):
    """Host-side shard: per-core input dicts for cores 0..7."""
    x = np.asarray(x, dtype=np.float32)
    w_qkv = np.asarray(w_qkv, dtype=np.float32)
    b_qkv = np.asarray(b_qkv, dtype=np.float32)
    w_out = np.asarray(w_out, dtype=np.float32)

    per_group = []
    for g in range(2):
        rows = np.concatenate(
            [
                w_qkv[qkv * DIM + g * FEAT : qkv * DIM + (g + 1) * FEAT]
                for qkv in range(3)
            ],
            axis=0,
        )  # [1152, 768]
        wqkvT_g = np.ascontiguousarray(rows.T).astype(_bf16)  # [768, 1152]
        b_rows = np.concatenate(
            [
                b_qkv[qkv * DIM + g * FEAT : qkv * DIM + (g + 1) * FEAT]
                for qkv in range(3)
            ],
            axis=0,
        )  # [1152]
        bias_g = np.ascontiguousarray(b_rows.reshape(9, 128).T)  # [128, 9]
        woutT_g = _round_to_f32r(w_out[:, g * FEAT : (g + 1) * FEAT].T)
        per_group.append((wqkvT_g, bias_g, woutT_g))

    xT_bf = [np.ascontiguousarray(x[b].T).astype(_bf16) for b in range(B)]
    in_maps = []
    for c in range(NCORES):
        b, g = c // 2, c % 2
        wqkvT_g, bias_g, woutT_g = per_group[g]
        in_maps.append(
            {
                "xT": xT_bf[b],
                "wqkvT": wqkvT_g,
                "bqkv": bias_g,
                "woutT": woutT_g,
            }
        )
    return in_maps


def assemble_output(results, b_out):
    """Host-side unshard: sum partials per batch pair, transpose, add bias."""
    b_out = np.asarray(b_out, dtype=np.float32)
    out = np.empty((B, N, DIM), dtype=np.float32)
    for b in range(B):
        pT = results[2 * b]["outT"] + results[2 * b + 1]["outT"]  # [768, 2048]
        out[b] = pT.T + b_out[None, :]
    return out


def kernel(x, w_qkv, b_qkv, w_out, b_out):
    from concourse.bass_utils import run_bass_kernel_spmd

    nc = _get_program()
    in_maps = make_core_inputs(x, w_qkv, b_qkv, w_out)
    res = run_bass_kernel_spmd(nc, in_maps, list(range(NCORES)))
    return assemble_output(res.results, b_out)


# revision 45
# speedup vs baseline: 1.4093x; 1.0165x over previous
"""Multi-head attention forward on 8 Trainium2 NeuronCores (Bass/Tile).

Problem: x[4, 2048, 768] -> qkv proj (w_qkv[2304, 768]) -> 12-head attention
(softmax((q k^T) * 768^-0.5)) -> out proj (w_out[768, 768]).

Sharding: core c handles batch b = c//2 and a group of 6 heads g = c%2
(tensor parallel over heads within a batch pair). Each core computes a
partial output (its heads' contribution through the row-sliced out
projection, transposed: [768, 2048]); the host sums the two partials per
batch, transposes back and adds b_out.

Device-side layout notes (everything transposed so the contraction dim sits
on SBUF partitions):
  xT   [768, 2048]  transposed on the HOST, plain DMA load (no PE transposes)
  qkvT [feat, 2048] = wT.T @ xT via bf16 matmuls
  qk_t [128, 4, N]: head A of the pair in rows 0:64 of chunks 0 (q) / 1 (k),
  head B in rows 64:128 of chunks 2/3; unused halves zeroed so the K=128
  scores contraction sees zero padding.
  scoresT[keys, q]  = kT_tile.T @ qT  (so attn@v needs no transpose)
  softmax without max-subtraction (scores are O(1); exp is safe in fp32);
  denominator comes free from an appended ones-column in v ("v_aug").
  Normalize: reciprocal_approx_fast on DVE, bf16 PE broadcast of the recip
  (deferred into the next block so it never stalls PE), DVE multiply.

Schedule: span-major blocks of 8 half-slots (2 score matmuls + 1 exp each);
attn@v trails its exp by LAG half-slots; qkv chains / v transposes for the
next head pair and out-proj tiles drain from a debt-paced filler queue so
the PE never idles while the ACT engine chews exps.
"""

import os
import sys

import ml_dtypes
import numpy as np

_bf16 = ml_dtypes.bfloat16

if "/opt/trn_rl_repo" not in sys.path:
    sys.path.insert(0, "/opt/trn_rl_repo")

B = 4
N = 2048
DIM = 768
HEADS = 12
DHEAD = 64
SCALE = DIM ** (-0.5)
NCORES = 8
HPC = 6  # heads per core
FEAT = HPC * DHEAD  # 384 per-core attention features

_PROGRAM = None  # (nc,) cached compiled bass program


def _build_program():
    from contextlib import ExitStack

    import concourse.bass as bass
    import concourse.tile as tile
    from concourse import bacc, mybir
    from concourse.masks import make_identity

    f32 = mybir.dt.float32
    f32r = mybir.dt.float32r
    bf16 = mybir.dt.bfloat16
    Alu = mybir.AluOpType
    ActF = mybir.ActivationFunctionType

    nc = bacc.Bacc("TRN2", target_bir_lowering=False, debug=False)

    x_in = nc.dram_tensor("xT", [DIM, N], bf16, kind="ExternalInput")
    wqkvT = nc.dram_tensor("wqkvT", [DIM, 3 * FEAT], bf16, kind="ExternalInput")
    bqkv = nc.dram_tensor("bqkv", [128, 9], f32, kind="ExternalInput")
    woutT = nc.dram_tensor("woutT", [FEAT, DIM], f32r, kind="ExternalInput")
    out_T = nc.dram_tensor("outT", [DIM, N], f32, kind="ExternalOutput")

    NT = N // 128  # 16 key tiles
    KC = DIM // 128  # 6 contraction chunks for dim
    NSPAN = N // 512  # 4 moving spans

    HALF_BUDGET = 400.0  # ns of filler debt accrued per half-slot
    DEBT_CAP = 3000.0  # bounds the backlog a single slot can release
    CHAIN_COST = 2600.0
    VTRANS_COST = 900.0
    VAUGI_COST = 1100.0
    OUTPROJ_COST = 750.0
    LAG = 3  # attnv trails its scores by this many half-slots

    with tile.TileContext(nc) as tc, ExitStack() as ctx:
        const = ctx.enter_context(tc.tile_pool(name="const", bufs=1))
        identity_bf = const.tile([128, 128], bf16)
        make_identity(nc, identity_bf)
        ones_f32 = const.tile([128, 1], f32)
        nc.vector.memset(ones_f32[:, :], 1.0)
        bias_sb = const.tile([128, 9], f32)
        nc.gpsimd.dma_start(bias_sb[:, :], bqkv[:, :])

        # ---- xT: transposed on host, plain DMA. Interleave xT/w chunks
        # across both queues so chain matmul j has xT[j]+w[j] early.
        xt_pool = ctx.enter_context(tc.tile_pool(name="xT", bufs=1))
        xT = xt_pool.tile([128, KC, N], bf16)
        wpool = ctx.enter_context(tc.tile_pool(name="w", bufs=1))
        w_all = wpool.tile([128, KC, 3 * FEAT], bf16)
        # xT chunks on gpsimd, weights on sync: chain matmul j needs
        # (xT[j], w[j]); issue cost stays low (6 DMAs per queue).
        nc.sync.dma_start(w_all[:, 0, :], wqkvT[0:128, :])
        for j in range(KC):
            eng = nc.gpsimd if j % 2 == 0 else nc.sync
            eng.dma_start(xT[:, j, :], x_in[j * 128 : (j + 1) * 128, :])
            if j > 0:
                nc.sync.dma_start(
                    w_all[:, j, :], wqkvT[j * 128 : (j + 1) * 128, :]
                )
        wout_sb = wpool.tile([128, 3, DIM], f32r)
        for c in range(3):
            nc.sync.dma_start(wout_sb[:, c, :], woutT[c * 128 : (c + 1) * 128, :])

        # PSUM: spool 3x[128,2,512] (6 banks) + opool 2x[128,512] (2 banks)
        spool = ctx.enter_context(tc.tile_pool(name="spsum", bufs=3, space="PSUM"))
        opool = ctx.enter_context(tc.tile_pool(name="opsum", bufs=2, space="PSUM"))

        qk_pool = ctx.enter_context(tc.tile_pool(name="qk", bufs=2))
        vt_pool = ctx.enter_context(tc.tile_pool(name="vt", bufs=2))
        vaug_pool = ctx.enter_context(tc.tile_pool(name="vaug", bufs=2))
        # deep: blocks 0-1 defer ALL their attnv pops to block 2+, so up to
        # ~18 exp tiles are simultaneously live at warmup
        exp_pool = ctx.enter_context(tc.tile_pool(name="expT", bufs=20))
        den8_pool = ctx.enter_context(tc.tile_pool(name="den8", bufs=2))
        rcp8_pool = ctx.enter_context(tc.tile_pool(name="rcp8", bufs=2))
        rcp8b_pool = ctx.enter_context(tc.tile_pool(name="rcp8b", bufs=2))
        rrow_pool = ctx.enter_context(tc.tile_pool(name="rrow", bufs=2))
        rbc_pool = ctx.enter_context(tc.tile_pool(name="rbc", bufs=2))
        sbo_pool = ctx.enter_context(tc.tile_pool(name="sbo", bufs=3))
        hst_pool = ctx.enter_context(tc.tile_pool(name="hstage", bufs=2))
        ao_pool = ctx.enter_context(tc.tile_pool(name="attnout", bufs=1))
        attn_outT = ao_pool.tile([128, 3, N], f32r)
        ost_pool = ctx.enter_context(tc.tile_pool(name="ostage", bufs=3))

        def emit_qkv_chain(qk_t, vT_t, idx, m, sp2):
            """One (m, span-pair): 12 matmuls + biased copyback."""
            ps = spool.tile([128, 2, 512], f32, tag="s", name="ps_qkv")
            for u in range(2):
                span = 2 * sp2 + u
                for j in range(KC):
                    nc.tensor.matmul(
                        ps[:, u, :],
                        w_all[:, j, m * 128 : (m + 1) * 128],
                        xT[:, j, span * 512 : (span + 1) * 512],
                        start=(j == 0),
                        stop=(j == KC - 1),
                    )
            cols = slice(sp2 * 1024, (sp2 + 1) * 1024)
            if idx < 2:
                # head A -> chunk idx rows 0:64; head B -> chunk idx+2 rows 64:128
                nc.vector.tensor_scalar(
                    qk_t[0:64, idx, cols].rearrange("p (a b) -> p a b", a=2),
                    ps[0:64, :, :],
                    bias_sb[0:64, m : m + 1],
                    None,
                    Alu.add,
                )
                nc.vector.tensor_scalar(
                    qk_t[64:128, idx + 2, cols].rearrange("p (a b) -> p a b", a=2),
                    ps[64:128, :, :],
                    bias_sb[64:128, m : m + 1],
                    None,
                    Alu.add,
                )
            else:
                nc.vector.tensor_scalar(
                    vT_t[:, cols].rearrange("p (a b) -> p a b", a=2),
                    ps[:, :, :],
                    bias_sb[:, m : m + 1],
                    None,
                    Alu.add,
                )

        def emit_zero_pads_qk(qk_t):
            # zero the unused halves so K padding contributes nothing.
            # Emitted ONCE per physical buffer (DVE, idle during DMA wait);
            # chain copybacks never touch the pad rows, so zeros persist
            # across the 2-buffer pair rotation.
            nc.vector.memset(qk_t[64:128, 0:2, :], 0.0)
            nc.vector.memset(qk_t[0:64, 2:4, :], 0.0)

        def emit_zero_pads_vaug(vaug_t):
            # once per buffer: zero all of vaug, then set the ones columns;
            # vtrans only rewrites data cols 0:64 of each half, so pad zeros
            # and ones survive pair rotation.
            nc.gpsimd.memset(vaug_t[:, :, :], 0.0)
            ones_cols = vaug_t[:, :, :].rearrange("p k (t c) -> p k t c", t=2)[
                :, :, :, 64:65
            ]
            nc.vector.tensor_copy(
                out=ones_cols, in_=ones_f32[:, :].to_broadcast((128, NT, 2, 1))
            )

        def emit_vtrans(vaug_t, vT_t, kc0, kcn):
            for kc in range(kc0, kcn):
                tp = spool.tile([128, 128], bf16, tag="s", name="tp_v")
                nc.tensor.transpose(
                    tp[:, :], vT_t[:, kc * 128 : (kc + 1) * 128], identity_bf[:, :]
                )
                nc.vector.tensor_copy(
                    out=vaug_t[:, kc, :].rearrange("p (t c) -> p t c", t=2)[
                        :, :, 0:64
                    ],
                    in_=tp[:, :].rearrange("p (t c) -> p t c", t=2),
                )

        def emit_vaug_init(vaug_t, vT_t):
            emit_vtrans(vaug_t, vT_t, 0, 4)

        def emit_outproj(m, span):
            """One out-proj tile [128, 512]: 3 matmuls + copyback + DMA."""
            ps = spool.tile([128, 2, 512], f32, tag="s", name="ps_op")
            for c in range(3):
                nc.tensor.matmul(
                    ps[:, 0, :],
                    wout_sb[:, c, m * 128 : (m + 1) * 128],
                    attn_outT[:, c, span * 512 : (span + 1) * 512],
                    start=(c == 0),
                    stop=(c == 2),
                )
            ostage = ost_pool.tile([128, 512], f32, name="ostage", tag="ostage")
            nc.vector.tensor_copy(out=ostage[:, :], in_=ps[:, 0, :])
            nc.gpsimd.dma_start(
                out_T[m * 128 : (m + 1) * 128, span * 512 : (span + 1) * 512],
                ostage[:, :],
            )

        # ---- filler machinery: (cost, closure) FIFO drained by debt ----
        filler_q = []
        debt = [3000.0]
        markers = set()  # emission-order guarantees (see pump_until)

        def pump(amount):
            # at most ONE filler per slot: spreads bursts (e.g. 6 outproj
            # landing at once must not all fire in one slot, starving ACT)
            debt[0] = min(debt[0] + amount, DEBT_CAP)
            if filler_q and debt[0] >= filler_q[0][0]:
                c, fn = filler_q.pop(0)
                fn()
                debt[0] -= c

        def pump_until(marker):
            """Force-drain fillers until `marker` has been emitted. Readers
            of filler-written tiles MUST be emitted after the writer."""
            while marker not in markers and filler_q:
                filler_q.pop(0)[1]()
            assert marker in markers, f"filler marker {marker} never queued"

        def marked(marker, fn):
            def go():
                fn()
                markers.add(marker)

            return go

        def make_pair_tiles():
            qk_t = qk_pool.tile([128, 4, N], bf16, name="qk", tag="qk")
            vT_t = vt_pool.tile([128, N], bf16, name="vT", tag="vT")
            vaug_t = vaug_pool.tile([128, NT, 256], bf16, name="vaug", tag="vaug")
            return qk_t, vT_t, vaug_t

        def push_pair_fillers(hp, qk_t, vT_t, vaug_t):
            """Queue pair hp's prep work as fillers for the prior pair."""
            for idx, m, sp2 in (
                (0, hp, 0),
                (1, 3 + hp, 0),
                (1, 3 + hp, 1),
                (2, 6 + hp, 0),
                (2, 6 + hp, 1),
            ):
                filler_q.append(
                    (
                        CHAIN_COST,
                        lambda i=idx, mm=m, s=sp2: emit_qkv_chain(qk_t, vT_t, i, mm, s),
                    )
                )
            filler_q.append(
                (
                    CHAIN_COST,
                    marked(
                        ("q1", hp),
                        lambda: emit_qkv_chain(qk_t, vT_t, 0, hp, 1),
                    ),
                )
            )
            filler_q.append((VAUGI_COST, lambda: emit_vaug_init(vaug_t, vT_t)))
            for kc0 in (4, 8, 12):
                fn = lambda k=kc0: emit_vtrans(vaug_t, vT_t, k, k + 4)
                if kc0 == 12:
                    fn = marked(("vtall", hp), fn)
                filler_q.append((VTRANS_COST, fn))

        # ---- attention block machinery ----
        pending = []  # closures popped with LAG half-slots of delay
        flush_done = {}  # hp=2 spans: both flushes seen -> outproj ready

        def mk_attnv(po, vaug_t, j, et, half):
            def go():
                for u in range(2):
                    kc = 2 * half + u
                    nc.tensor.matmul(
                        po[:, :],
                        vaug_t[:, kc, j * 128 : (j + 1) * 128],
                        et[:, u, :],
                        start=(kc == 0),
                        stop=(kc == 15),
                    )

            return go

        def emit_block(hp, j, span, qk_t, vaug_t, carry_flush, lag=LAG, mid=None):
            po = opool.tile([128, 512], f32, tag="o", name="po")
            qT = qk_t[:, 2 * j, span * 512 : (span + 1) * 512]
            for half in range(8):
                if half == 4 and mid is not None:
                    mid()
                    mid = None
                ps = spool.tile([128, 2, 512], f32, tag="s", name="ps_s")
                for u in range(2):
                    kc = 2 * half + u
                    nc.tensor.matmul(
                        ps[:, u, :],
                        qk_t[:, 2 * j + 1, kc * 128 : (kc + 1) * 128],
                        qT,
                        start=True,
                        stop=True,
                    )
                et = exp_pool.tile([128, 2, 512], bf16)
                nc.scalar.activation(
                    et[:, :, :], ps[:, :, :], ActF.Exp, scale=float(SCALE)
                )
                pending.append(mk_attnv(po, vaug_t, j, et, half))
                if half == 6 and carry_flush is not None:
                    pending.append(carry_flush)
                    carry_flush = None
                pump(HALF_BUDGET)
                npop = 0
                while len(pending) > lag and npop < 3:
                    pending.pop(0)()
                    npop += 1
            # post-block: snapshot numerators+denominator; reciprocal is
            # lane-spread: scatter the 512 denominators over 64 partitions
            # via SBUF DMA so the DVE reciprocal runs 64-wide (~0.2us, not
            # 3.3us — the serial [1,512] version plugged the DVE queue and
            # stalled spool rotation behind it).
            sb_o = sbo_pool.tile([65, 512], f32, name="sb_o")
            den8 = den8_pool.tile([64, 8], f32, name="den8")
            rcp8 = rcp8_pool.tile([64, 8], f32, name="rcp8")
            rcp8b = rcp8b_pool.tile([64, 8], bf16, name="rcp8b")
            rcp_row = rrow_pool.tile([1, 512], bf16, name="rcp_row")
            rcp_bc = rbc_pool.tile([64, 512], bf16, name="rcp_bc")

            def post_dve(po=po, sb_o=sb_o, den8=den8, rcp8=rcp8, rcp8b=rcp8b,
                         rcp_row=rcp_row, rcp_bc=rcp_bc):
                nc.vector.tensor_copy(out=sb_o[:, :], in_=po[0:65, :])
                nc.sync.dma_start(den8[:, :], sb_o[64:65, :])
                nc.vector.reciprocal(rcp8[:, :], den8[:, :])
                nc.vector.tensor_copy(out=rcp8b[:, :], in_=rcp8[:, :])
                nc.sync.dma_start(rcp_row[0:1, :], rcp8b[:, :])
                # gpsimd partition-broadcast: replicate the recip row to 64
                # partitions (frees the PE of 24 broadcast matmuls and
                # releases po as soon as the snapshot copy is done)
                nc.gpsimd.partition_broadcast(rcp_bc[:, :], rcp_row[0:1, :])

            pending.append(post_dve)

            def flush(sb_o=sb_o, rcp_bc=rcp_bc, hp=hp, j=j, span=span):
                if j == 0:
                    ddst = attn_outT[0:64, hp, span * 512 : (span + 1) * 512]
                else:
                    ddst = hst_pool.tile([64, 512], f32r, name="hstage", tag="hstage")
                nc.vector.tensor_tensor(
                    out=ddst, in0=sb_o[0:64, :], in1=rcp_bc[0:64, :], op=Alu.mult
                )
                if j == 1:
                    nc.sync.dma_start(
                        attn_outT[64:128, hp, span * 512 : (span + 1) * 512],
                        ddst[:, :],
                    )
                if hp == 2:
                    flush_done[span] = flush_done.get(span, 0) + 1
                    if flush_done[span] == 2:
                        for m in range(DIM // 128):
                            filler_q.append(
                                (OUTPROJ_COST, lambda mm=m, s=span: emit_outproj(mm, s))
                            )

            return flush

        # ---- prologue: the minimum for the first scores block: q sp0 +
        # both k chains. v/vaug/q-sp1 drain as fillers during block 0,
        # whose attnv+normalize are deferred wholesale into block 1.
        # Both physical buffer sets are allocated up front and their pads
        # zeroed exactly once (they persist across pair rotation).
        tilesets = [make_pair_tiles(), make_pair_tiles()]
        qk0, vT0, vaug0 = tilesets[0]
        emit_zero_pads_qk(tilesets[0][0])
        emit_zero_pads_qk(tilesets[1][0])
        emit_zero_pads_vaug(tilesets[0][2])
        emit_zero_pads_vaug(tilesets[1][2])
        emit_qkv_chain(qk0, vT0, 0, 0, 0)  # q sp0
        emit_qkv_chain(qk0, vT0, 1, 3, 0)  # k sp0 (k sp1 lands mid-block-0)
        filler_q.append(
            (CHAIN_COST, lambda: emit_qkv_chain(qk0, vT0, 2, 6, 0))  # v sp0
        )
        filler_q.append(
            (CHAIN_COST, lambda: emit_qkv_chain(qk0, vT0, 2, 6, 1))  # v sp1
        )
        filler_q.append((VAUGI_COST, lambda: emit_vaug_init(vaug0, vT0)))
        for kc0 in (4, 8, 12):
            fn = lambda k=kc0: emit_vtrans(vaug0, vT0, k, k + 4)
            if kc0 == 12:
                fn = marked(("vtall", 0), fn)
            filler_q.append((VTRANS_COST, fn))
        filler_q.append(
            (
                CHAIN_COST,
                marked(("q1", 0), lambda: emit_qkv_chain(qk0, vT0, 0, 0, 1)),
            )
        )

        # ---- attention: 3 pairs x 4 spans x 2 heads, span-major ----
        carry = None
        blk = 0
        for hp in range(3):
            qk_t, vT_t, vaug_t = tilesets[hp % 2]
            if hp < 2:
                push_pair_fillers(hp + 1, *tilesets[(hp + 1) % 2])
            for span in range(NSPAN):
                # last span of the last pair runs j=1 first so its hstage
                # DMA overlaps the j=0 block, shortening the serial tail
                js = (1, 0) if (hp == 2 and span == 3) else (0, 1)
                for j in js:
                    if span == 0 and j == js[0] and hp > 0:
                        pump_until(("vtall", hp))  # emission-order guard
                    if blk == 2:
                        pump_until(("vtall", 0))
                    if span == 2 and j == js[0]:
                        pump_until(("q1", hp))
                    # first two blocks: defer ALL attnv (their vaug is
                    # still being filled by fillers); drain paced later.
                    # Block 0 halves 0-3 only need k-sp0; k-sp1 lands mid.
                    lag = 99 if blk < 2 else LAG
                    mid = (
                        (lambda: emit_qkv_chain(qk0, vT0, 1, 3, 1))
                        if blk == 0
                        else None
                    )
                    carry = emit_block(
                        hp, j, span, qk_t, vaug_t, carry, lag=lag, mid=mid
                    )
                    blk += 1
        while pending:
            pending.pop(0)()
        carry()
        while filler_q:
            filler_q.pop(0)[1]()

    nc.compile()
    return nc


def _get_program():
    global _PROGRAM
    if _PROGRAM is None:
        _PROGRAM = _build_program()
    return _PROGRAM


def _round_to_f32r(a):
    """Round fp32 to the PE's fp32r format: 11-bit mantissa, low 12 bits zero
    (round to nearest, ties away handled approximately via +0x7FF + lsb)."""
    u = np.ascontiguousarray(a, dtype=np.float32).view(np.uint32)
    r = u + np.uint32(0x7FF) + ((u >> np.uint32(12)) & np.uint32(1))
    r &= np.uint32(0xFFFFF000)
    return r.view(np.float32)


def make_core_inputs(x, w_qkv, b_qkv, w_out):
    """Host-side shard: per-core input dicts for cores 0..7."""
    x = np.asarray(x, dtype=np.float32)
    w_qkv = np.asarray(w_qkv, dtype=np.float32)
    b_qkv = np.asarray(b_qkv, dtype=np.float32)
    w_out = np.asarray(w_out, dtype=np.float32)

    per_group = []
    for g in range(2):
        rows = np.concatenate(
            [
                w_qkv[qkv * DIM + g * FEAT : qkv * DIM + (g + 1) * FEAT]
                for qkv in range(3)
            ],
            axis=0,
        )  # [1152, 768]
        wqkvT_g = np.ascontiguousarray(rows.T).astype(_bf16)  # [768, 1152]
        b_rows = np.concatenate(
            [
                b_qkv[qkv * DIM + g * FEAT : qkv * DIM + (g + 1) * FEAT]
                for qkv in range(3)
            ],
            axis=0,
        )  # [1152]
        bias_g = np.ascontiguousarray(b_rows.reshape(9, 128).T)  # [128, 9]
        woutT_g = _round_to_f32r(w_out[:, g * FEAT : (g + 1) * FEAT].T)
        per_group.append((wqkvT_g, bias_g, woutT_g))

    xT_bf = [np.ascontiguousarray(x[b].T).astype(_bf16) for b in range(B)]
    in_maps = []
    for c in range(NCORES):
        b, g = c // 2, c % 2
        wqkvT_g, bias_g, woutT_g = per_group[g]
        in_maps.append(
            {
                "xT": xT_bf[b],
                "wqkvT": wqkvT_g,
                "bqkv": bias_g,
                "woutT": woutT_g,
            }
        )
    return in_maps


def assemble_output(results, b_out):
    """Host-side unshard: sum partials per batch pair, transpose, add bias."""
    b_out = np.asarray(b_out, dtype=np.float32)
    out = np.empty((B, N, DIM), dtype=np.float32)
    for b in range(B):
        pT = results[2 * b]["outT"] + results[2 * b + 1]["outT"]  # [768, 2048]
        out[b] = pT.T + b_out[None, :]
    return out


def kernel(x, w_qkv, b_qkv, w_out, b_out):
    from concourse.bass_utils import run_bass_kernel_spmd

    nc = _get_program()
    in_maps = make_core_inputs(x, w_qkv, b_qkv, w_out)
    res = run_bass_kernel_spmd(nc, in_maps, list(range(NCORES)))
    return assemble_output(res.results, b_out)
